# revision 1
# baseline (speedup 1.0000x reference)
"""GATv2 kernel v2: host stages per-edge-slot transposed features (src and
dst); the device computes xs = x_e@Wl + x_r@Wr + ea*We per 128-edge subtile
directly in PSUM (fp32r matmuls), so there are no indirect gathers and no
feature tables. Aggregation: one-hot S matmul into a per-window PSUM slab.
"""

import numpy as np
from contextlib import ExitStack

import concourse.bass as bass
import concourse.tile as tile
from concourse import bacc, mybir
from concourse.masks import make_identity

F32 = mybir.dt.float32
F32R = mybir.dt.float32r
I32 = mybir.dt.int32
P = 128
NEG = 0.2


def preprocess(x, edge_index, edge_attr, Wl, Wr, We, att, bias, n_cores):
    x = np.ascontiguousarray(np.asarray(x, np.float32))
    src = np.asarray(edge_index[0]).astype(np.int64)
    dst = np.asarray(edge_index[1]).astype(np.int64)
    ea = np.asarray(edge_attr, np.float32).reshape(-1)
    Wl = np.ascontiguousarray(np.asarray(Wl, np.float32))
    Wr = np.ascontiguousarray(np.asarray(Wr, np.float32))
    We = np.asarray(We, np.float32).reshape(-1)
    att = np.asarray(att, np.float32)
    bias = np.asarray(bias, np.float32).reshape(-1)

    N, F = x.shape
    HC = Wl.shape[1]
    E = src.shape[0]
    assert F == P
    assert N % n_cores == 0
    ND = N // n_cores
    W = (ND + P - 1) // P
    NDpad = W * P

    cnt = np.bincount(dst, minlength=N).astype(np.int64)
    invc_full = (1.0 / np.maximum(cnt, 1)).astype(np.float32)

    order = np.argsort(dst, kind="stable")
    src_s, dst_s, ea_s = src[order], dst[order], ea[order]

    core = dst_s // ND
    loc = dst_s % ND
    w_of = loc // P
    dl = (loc % P).astype(np.float32)
    key = core * W + w_of
    counts = np.bincount(key, minlength=n_cores * W)
    T = 1 + int(np.ceil(max(counts.max(), 1) / P))
    starts = np.zeros(n_cores * W, np.int64)
    np.cumsum(counts[:-1], out=starts[1:])
    rank = np.arange(E) - starts[key]
    t_of = rank // P
    p_of = rank % P

    # per-slot node ids (src / dst), default 0
    slot_src = np.zeros((n_cores, W, T, P), np.int64)
    slot_dst = np.zeros((n_cores, W, T, P), np.int64)
    edstl = np.zeros((n_cores, W, P, T), np.float32)
    emask = np.full((n_cores, W, P, T), -1e4, np.float32)
    eacol = np.zeros((n_cores, W, P, T), np.float32)
    invcnt = np.ones((n_cores, W, P), np.float32)

    slot_src[core, w_of, t_of, p_of] = src_s
    slot_dst[core, w_of, t_of, p_of] = dst_s
    edstl[core, w_of, p_of, t_of] = dl
    emask[core, w_of, p_of, t_of] = 0.0
    eacol[core, w_of, p_of, t_of] = ea_s

    # self-loop subtile t=T-1
    gid = np.arange(NDpad)
    for c in range(n_cores):
        g = c * ND + gid
        valid = gid < ND
        gsafe = np.where(valid, g, 0)
        slot_src[c, :, T - 1, :] = gsafe.reshape(W, P)
        slot_dst[c, :, T - 1, :] = gsafe.reshape(W, P)
        edstl[c, :, :, T - 1] = np.tile(np.arange(P, dtype=np.float32), W).reshape(W, P)
        emask[c, :, :, T - 1] = 0.0
        eacol[c, :, :, T - 1] = 0.0
        invcnt[c] = np.where(valid, invc_full[gsafe], 1.0).reshape(W, P)

    earow = np.ascontiguousarray(
        eacol.transpose(0, 1, 3, 2).reshape(n_cores, W, T * P)
    )
    import ml_dtypes
    # one-hot S per slot: [n_cores, W, P, T*128] bf16 (exact 0/1)
    s_all = (
        edstl[..., None] == np.arange(P, dtype=np.float32)
    ).astype(np.float32)
    s_all = np.ascontiguousarray(s_all.reshape(n_cores, W, P, T * P))
    eacol_bf = eacol

    xT = x.T  # [F, N]
    att_rep = np.ascontiguousarray(np.broadcast_to(att.reshape(1, HC), (P, HC)))
    bias_rep = np.ascontiguousarray(np.broadcast_to(bias.reshape(1, HC), (P, HC)))
    we_rep = np.ascontiguousarray(np.broadcast_to(We.reshape(1, HC), (P, HC)))

    in_maps = []
    for c in range(n_cores):
        flat_s = slot_src[c].reshape(-1)  # [W*T*P] slot-major
        flat_d = slot_dst[c].reshape(-1)
        xTe = np.ascontiguousarray(xT[:, flat_s])  # [128, W*T*128]
        xTr = np.ascontiguousarray(xT[:, flat_d])
        in_maps.append(
            dict(
                xTe=xTe,
                xTr=xTr,
                Wl=Wl,
                Wr=Wr,
                we_rep=we_rep,
                att_rep=att_rep,
                bias_rep=bias_rep,
                s_all=s_all[c],
                emask=np.ascontiguousarray(emask[c]),
                eacol=np.ascontiguousarray(eacol_bf[c]),
                earow=np.ascontiguousarray(earow[c]),
                invcnt=np.ascontiguousarray(invcnt[c]),
            )
        )
    meta = dict(W=W, T=T, HC=HC, ND=ND, NDpad=NDpad, n_cores=n_cores)
    return in_maps, meta


def build(meta, s_on_pool=False):
    W, T, HC = meta["W"], meta["T"], meta["HC"]
    NDpad = meta["NDpad"]
    H = HC // 32
    WT = W * T

    nc = bacc.Bacc("TRN2", target_bir_lowering=False, debug=False)

    xTe = nc.dram_tensor("xTe", [P, WT * P], F32R, kind="ExternalInput")
    xTr = nc.dram_tensor("xTr", [P, WT * P], F32R, kind="ExternalInput")
    Wl = nc.dram_tensor("Wl", [P, HC], F32R, kind="ExternalInput")
    Wr = nc.dram_tensor("Wr", [P, HC], F32R, kind="ExternalInput")
    we_rep = nc.dram_tensor("we_rep", [P, HC], F32R, kind="ExternalInput")
    att_rep = nc.dram_tensor("att_rep", [P, HC], F32, kind="ExternalInput")
    bias_rep = nc.dram_tensor("bias_rep", [P, HC], F32, kind="ExternalInput")
    s_all = nc.dram_tensor("s_all", [W, P, T * P], F32R, kind="ExternalInput")
    emask = nc.dram_tensor("emask", [W, P, T], F32, kind="ExternalInput")
    eacol = nc.dram_tensor("eacol", [W, P, T], F32R, kind="ExternalInput")
    earow = nc.dram_tensor("earow", [W, T * P], F32R, kind="ExternalInput")
    invcnt = nc.dram_tensor("invcnt", [W, P], F32, kind="ExternalInput")
    out = nc.dram_tensor("out", [NDpad, HC], F32, kind="ExternalOutput")

    with tile.TileContext(nc) as tc, ExitStack() as ctx:
        cpool = ctx.enter_context(tc.tile_pool(name="cpool", bufs=1))
        wl_t = cpool.tile([P, HC], F32R)
        nc.sync.dma_start(wl_t[:], Wl[:, :])
        wr_t = cpool.tile([P, HC], F32R)
        nc.sync.dma_start(wr_t[:], Wr[:, :])
        werep_t = cpool.tile([P, HC], F32R)
        nc.sync.dma_start(werep_t[:], we_rep[:, :])
        attrep_t = cpool.tile([P, HC], F32)
        nc.sync.dma_start(attrep_t[:], att_rep[:, :])
        attrep2_t = cpool.tile([P, 2 * HC], F32)
        nc.sync.dma_start(attrep2_t[:, 0:HC], att_rep[:, :])
        nc.sync.dma_start(attrep2_t[:, HC : 2 * HC], att_rep[:, :])
        attrep4_t = cpool.tile([P, 4 * HC], F32)
        for _r in range(4):
            nc.sync.dma_start(attrep4_t[:, _r * HC : (_r + 1) * HC], att_rep[:, :])
        biasrep_t = cpool.tile([P, HC], F32)
        nc.sync.dma_start(biasrep_t[:], bias_rep[:, :])
        iota_t = cpool.tile([P, P], F32)
        nc.gpsimd.iota(
            iota_t[:],
            pattern=[[1, P]],
            base=0,
            channel_multiplier=0,
            allow_small_or_imprecise_dtypes=True,
        )

        with tc.tile_pool(name="win", bufs=2) as winp, tc.tile_pool(
            name="sub", bufs=4
        ) as subp, tc.tile_pool(name="xsps", bufs=2, space="PSUM") as xsps, tc.tile_pool(
            name="xlps", bufs=4, space="PSUM"
        ) as xlps, tc.tile_pool(name="aggps", bufs=1, space="PSUM") as aggps, tc.tile_pool(
            name="easps", bufs=1, space="PSUM"
        ) as easps:
            for w in range(W):
                S_w_hnd = winp.tile([P, T * P], F32R)
                nc.sync.dma_start(S_w_hnd[:], s_all[w, :, :])
                emask_t = winp.tile([P, T], F32)
                nc.sync.dma_start(emask_t[:], emask[w, :, :])
                eacol_t = winp.tile([P, T], F32R)
                nc.sync.dma_start(eacol_t[:], eacol[w, :, :])
                earow_t = winp.tile([1, T * P], F32R)
                nc.sync.dma_start(earow_t[:], earow[w : w + 1, :])
                invc_t = winp.tile([P, 1], F32)
                nc.sync.dma_start(invc_t[:], invcnt[w, :, None])

                agg_ps = aggps.tile([P, H + HC], F32)
                easum_ps = easps.tile([P, 2], F32)
                xe_w = winp.tile([P, T * P], F32R)
                nc.sync.dma_start(xe_w[:], xTe[:, w * T * P : (w + 1) * T * P])
                xr_w = winp.tile([P, T * P], F32R)
                nc.sync.dma_start(xr_w[:], xTr[:, w * T * P : (w + 1) * T * P])

                n_pair = max((T - 2) // 2, 0)
                n_quad = n_pair // 2

                def _pair_phaseA(t0):
                    xs_pair = xsps.tile([P, 2 * HC], F32, tag="xs")
                    xl_pair = xlps.tile([P, 2 * HC], F32, tag="xl")
                    for h in range(2):
                        t = t0 + h
                        xe_t = xe_w[:, t * P : (t + 1) * P]
                        xr_t = xr_w[:, t * P : (t + 1) * P]
                        reg = xs_pair[:, h * HC : (h + 1) * HC]
                        nc.tensor.matmul(
                            xl_pair[:, h * HC : (h + 1) * HC], xe_t, wl_t[:],
                            start=(h == 0), stop=(h == 1),
                        )
                        nc.tensor.matmul(
                            reg,
                            earow_t[0:1, t * P : (t + 1) * P],
                            werep_t[0:1, :],
                            start=(h == 0), stop=False,
                        )
                        nc.tensor.matmul(reg, xe_t, wl_t[:], start=False, stop=False)
                        nc.tensor.matmul(
                            reg, xr_t, wr_t[:], start=False, stop=(h == 1)
                        )
                    return xs_pair, xl_pair

                def _pair_phaseC(t0, xl_pair, lg, lgoff):
                    exY2 = subp.tile([P, 2 * (H + HC)], F32R, tag="exY2")
                    exY2v = exY2[:].rearrange("p (u q) -> p u q", u=2)
                    for h in range(2):
                        nc.scalar.activation(
                            out=exY2v[:, h : h + 1, 0:H],
                            in_=lg[:, (lgoff + h) * H : (lgoff + h + 1) * H].unsqueeze(1),
                            func=mybir.ActivationFunctionType.Exp,
                            bias=emask_t[:, t0 + h : t0 + h + 1],
                            scale=1.0,
                        )
                    nc.vector.tensor_tensor(
                        out=exY2v[:, :, H : H + HC].rearrange(
                            "p u (h c) -> p u h c", c=32
                        ),
                        in0=xl_pair[:].rearrange("p (u q) -> p u q", u=2).rearrange(
                            "p u (h c) -> p u h c", c=32
                        ),
                        in1=exY2v[:, :, 0:H].bitcast(F32).unsqueeze(3).to_broadcast(
                            [P, 2, H, 32]
                        ),
                        op=mybir.AluOpType.mult,
                    )
                    for h in range(2):
                        t = t0 + h
                        nc.tensor.matmul(
                            agg_ps[:], S_w_hnd[:, t * P : (t + 1) * P],
                            exY2[:, h * (H + HC) : (h + 1) * (H + HC)],
                            start=(t == 0), stop=False,
                        )
                        nc.tensor.matmul(
                            easum_ps[:],
                            S_w_hnd[:, t * P : (t + 1) * P],
                            eacol_t[:, t : t + 2],
                            start=(t == 0), stop=(t == T - 2),
                        )

                for qi in range(n_quad):
                    t0a, t0b = 4 * qi, 4 * qi + 2
                    xs_a, xl_a = _pair_phaseA(t0a)
                    xs_b, xl_b = _pair_phaseA(t0b)
                    xs_act4 = subp.tile([P, 4 * HC], F32, tag="xsact4")
                    nc.scalar.activation(
                        out=xs_act4[:, 0 : 2 * HC], in_=xs_a[:],
                        func=mybir.ActivationFunctionType.Prelu,
                        bias=0.0, scale=1.0, alpha=NEG,
                    )
                    nc.scalar.activation(
                        out=xs_act4[:, 2 * HC : 4 * HC], in_=xs_b[:],
                        func=mybir.ActivationFunctionType.Prelu,
                        bias=0.0, scale=1.0, alpha=NEG,
                    )
                    tm4 = subp.tile([P, 4 * HC], F32, tag="tm4")
                    nc.vector.tensor_mul(out=tm4[:], in0=xs_act4[:], in1=attrep4_t[:])
                    lg4 = subp.tile([P, 4 * H], F32, tag="lg4")
                    nc.vector.tensor_reduce(
                        out=lg4[:],
                        in_=tm4[:].rearrange("p (h c) -> p h c", c=32),
                        axis=mybir.AxisListType.X,
                        op=mybir.AluOpType.add,
                    )
                    _pair_phaseC(t0a, xl_a, lg4, 0)
                    _pair_phaseC(t0b, xl_b, lg4, 2)
                for pi in range(2 * n_quad, n_pair):
                    t0 = 2 * pi
                    xs_pair, xl_pair = _pair_phaseA(t0)
                    xs_act2 = subp.tile([P, 2 * HC], F32, tag="xsact2")
                    nc.scalar.activation(
                        out=xs_act2[:], in_=xs_pair[:],
                        func=mybir.ActivationFunctionType.Prelu,
                        bias=0.0, scale=1.0, alpha=NEG,
                    )
                    tm2 = subp.tile([P, 2 * HC], F32, tag="tm2")
                    nc.vector.tensor_mul(out=tm2[:], in0=xs_act2[:], in1=attrep2_t[:])
                    lg2 = subp.tile([P, 2 * H], F32, tag="lg2")
                    nc.vector.tensor_reduce(
                        out=lg2[:],
                        in_=tm2[:].rearrange("p (h c) -> p h c", c=32),
                        axis=mybir.AxisListType.X,
                        op=mybir.AluOpType.add,
                    )
                    _pair_phaseC(t0, xl_pair, lg2, 0)
                for t in range(2 * n_pair, T):
                    is_self = t == T - 1
                    S_t = S_w_hnd[:, t * P : (t + 1) * P]
                    xe_t = xe_w[:, t * P : (t + 1) * P]
                    xr_t = xr_w[:, t * P : (t + 1) * P]

                    xs_ps = xsps.tile([P, HC], F32, tag='xs')
                    xl_ps = xlps.tile([P, HC], F32, tag='xl')
                    # xl for the Y path (same stationary as the xs xl-matmul)
                    nc.tensor.matmul(
                        xl_ps[:], xe_t, wl_t[:], start=True, stop=True
                    )
                    if not is_self:
                        nc.tensor.matmul(
                            xs_ps[:],
                            earow_t[0:1, t * P : (t + 1) * P],
                            werep_t[0:1, :],
                            start=True, stop=False,
                        )
                        nc.tensor.matmul(
                            xs_ps[:], xe_t, wl_t[:], start=False, stop=False
                        )
                        nc.tensor.matmul(
                            xs_ps[:], xr_t, wr_t[:], start=False, stop=True
                        )
                        xs_in = xs_ps
                    else:
                        la = subp.tile([P, 1], F32)
                        nc.vector.tensor_mul(
                            out=la[:], in0=easum_ps[:, 0:1], in1=invc_t[:]
                        )
                        nc.tensor.matmul(
                            xs_ps[:], xe_t, wl_t[:], start=True, stop=False
                        )
                        nc.tensor.matmul(
                            xs_ps[:], xr_t, wr_t[:], start=False, stop=True
                        )
                        xs_pre = subp.tile([P, HC], F32)
                        nc.vector.scalar_tensor_tensor(
                            out=xs_pre[:],
                            in0=werep_t[:].bitcast(F32),
                            scalar=la[:, 0:1],
                            in1=xs_ps[:],
                            op0=mybir.AluOpType.mult,
                            op1=mybir.AluOpType.add,
                        )
                        xs_in = xs_pre
                    xs_act = subp.tile([P, HC], F32)
                    nc.scalar.activation(
                        out=xs_act[:],
                        in_=xs_in[:],
                        func=mybir.ActivationFunctionType.Prelu,
                        bias=0.0,
                        scale=1.0,
                        alpha=NEG,
                    )
                    tm = subp.tile([P, HC], F32)
                    nc.vector.tensor_mul(out=tm[:], in0=xs_act[:], in1=attrep_t[:])
                    lg = subp.tile([P, H], F32)
                    nc.vector.tensor_reduce(
                        out=lg[:],
                        in_=tm[:].rearrange("p (h c) -> p h c", c=32),
                        axis=mybir.AxisListType.X,
                        op=mybir.AluOpType.add,
                    )
                    exY = subp.tile([P, H + HC], F32R)
                    nc.scalar.activation(
                        out=exY[:, 0:H],
                        in_=lg[:],
                        func=mybir.ActivationFunctionType.Exp,
                        bias=emask_t[:, t : t + 1],
                        scale=1.0,
                    )
                    nc.vector.tensor_tensor(
                        out=exY[:, H : H + HC].rearrange("p (h c) -> p h c", c=32),
                        in0=xl_ps[:].rearrange("p (h c) -> p h c", c=32),
                        in1=exY[:, 0:H].bitcast(F32).unsqueeze(2).to_broadcast(
                            [P, H, 32]
                        ),
                        op=mybir.AluOpType.mult,
                    )
                    nc.tensor.matmul(
                        agg_ps[:], S_t, exY[:],
                        start=(t == 0), stop=(t == T - 1),
                    )
                    if not is_self:
                        nc.tensor.matmul(
                            easum_ps[:],
                            S_t,
                            eacol_t[:, t : t + 2],
                            start=(t == 0), stop=(t == T - 2),
                        )

                rc = subp.tile([P, H], F32)
                nc.vector.reciprocal(rc[:], agg_ps[:, 0:H])
                ow = subp.tile([P, HC], F32)
                nc.vector.tensor_tensor(
                    out=ow[:].rearrange("p (h c) -> p h c", c=32),
                    in0=agg_ps[:, H : H + HC].rearrange("p (h c) -> p h c", c=32),
                    in1=rc[:].unsqueeze(2).to_broadcast([P, H, 32]),
                    op=mybir.AluOpType.mult,
                )
                ow2 = subp.tile([P, HC], F32)
                nc.vector.tensor_add(out=ow2[:], in0=ow[:], in1=biasrep_t[:])
                nc.sync.dma_start(out[w * P : (w + 1) * P, :], ow2[:])

    nc.compile()
    return nc


_LAST_RESULT = None


def kernel(**inputs):
    """Full-input GATv2 forward on 8 TRN2 NeuronCores (dst-sharded)."""
    global _LAST_RESULT
    n_cores = 8
    in_maps, meta = preprocess(
        inputs["x"],
        inputs["edge_index"],
        inputs["edge_attr"],
        inputs["Wl"],
        inputs["Wr"],
        inputs["We"],
        inputs["att"],
        inputs["bias"],
        n_cores,
    )
    nc = build(meta)
    from concourse.bass_utils import run_bass_kernel_spmd

    res = run_bass_kernel_spmd(nc, in_maps, core_ids=list(range(n_cores)))
    _LAST_RESULT = res
    ND = meta["ND"]
    out = np.concatenate(
        [np.asarray(res.results[c]["out"])[:ND] for c in range(n_cores)], axis=0
    )
    return np.ascontiguousarray(out.astype(np.float32))



# revision 2
# speedup vs baseline: 1.3560x; 1.3560x over previous
"""GATv2 kernel v4: software-pipelined pair stream.

Same math/staging as v3 (merged self-loops, LPT-balanced bins, bf16 tables,
sum(alpha)=1 correction) but restructured as one flat stream of subtile
pairs with stage offsets so every engine stays busy:
  step i:  PE z(i) | DVE exY(i-3) | PE scatter(i-3) | Act lrelu(i-1)
           | Pool att-mult(i-1) | DVE head-reduce(i-2) | Act exp(i-2)
Padding slots are masked by zeros in the S/Sea scatter tables (no vmask).
"""

import numpy as np
from contextlib import ExitStack

import concourse.bass as bass
import concourse.tile as tile
from concourse import bacc, mybir

F32 = mybir.dt.float32
BF16 = mybir.dt.bfloat16
P = 128
NEG = 0.2


def preprocess(x, edge_index, edge_attr, Wl, Wr, We, att, bias, n_cores):
    import ml_dtypes

    BF = ml_dtypes.bfloat16
    x = np.ascontiguousarray(np.asarray(x, np.float32))
    src = np.asarray(edge_index[0]).astype(np.int64)
    dst = np.asarray(edge_index[1]).astype(np.int64)
    ea = np.asarray(edge_attr, np.float32).reshape(-1)
    Wl = np.ascontiguousarray(np.asarray(Wl, np.float32))
    Wr = np.ascontiguousarray(np.asarray(Wr, np.float32))
    We = np.asarray(We, np.float32).reshape(-1)
    att = np.asarray(att, np.float32)
    bias = np.asarray(bias, np.float32).reshape(-1)

    N, F = x.shape
    HC = Wl.shape[1]
    H = HC // 32
    assert F == P
    ND = N // n_cores
    W = (ND + P - 1) // P
    NB = n_cores * W

    cnt = np.bincount(dst, minlength=N).astype(np.int64)
    sums = np.zeros(N, np.float64)
    np.add.at(sums, dst, ea)
    la = (sums / np.maximum(cnt, 1)).astype(np.float32)

    src_f = np.concatenate([src, np.arange(N, dtype=np.int64)])
    dst_f = np.concatenate([dst, np.arange(N, dtype=np.int64)])
    ea_f = np.concatenate([ea, la])
    deg = cnt + 1

    import heapq

    order = np.argsort(-deg, kind="stable")
    heap = [(0, b) for b in range(NB)]
    heapq.heapify(heap)
    slots_used = np.zeros(NB, np.int64)
    node_bin = np.zeros(N, np.int64)
    node_row = np.zeros(N, np.int64)
    stash = []
    for n in order:
        while True:
            load, b = heapq.heappop(heap)
            if slots_used[b] < P:
                break
            stash.append((load, b))
        node_bin[n] = b
        node_row[n] = slots_used[b]
        slots_used[b] += 1
        heapq.heappush(heap, (load + int(deg[n]), b))
        for it in stash:
            heapq.heappush(heap, it)
        stash.clear()

    binload = np.bincount(node_bin[dst_f], minlength=NB)
    T = int(np.ceil(binload.max() / P))

    ebin = node_bin[dst_f]
    eorder = np.argsort(ebin, kind="stable")
    src_s = src_f[eorder]
    dst_s = dst_f[eorder]
    ea_s = ea_f[eorder]
    ebin_s = ebin[eorder]
    starts = np.zeros(NB, np.int64)
    np.cumsum(binload[:-1], out=starts[1:])
    rank = np.arange(len(src_s)) - starts[ebin_s]
    t_of = rank // P
    p_of = rank % P

    core = ebin_s // W
    w_of = ebin_s % W

    slot_src = np.zeros((n_cores, W, T, P), np.int64)
    slot_dstn = np.zeros((n_cores, W, T, P), np.int64)
    slot_row = np.zeros((n_cores, W, T, P), np.int64)
    slot_ea = np.zeros((n_cores, W, T, P), np.float32)
    slot_valid = np.zeros((n_cores, W, T, P), np.float32)

    slot_src[core, w_of, t_of, p_of] = src_s
    slot_dstn[core, w_of, t_of, p_of] = dst_s
    slot_row[core, w_of, t_of, p_of] = node_row[dst_s]
    slot_ea[core, w_of, t_of, p_of] = ea_s
    slot_valid[core, w_of, t_of, p_of] = 1.0

    rows = np.arange(P).reshape(1, 1, 1, 1, P)
    onehot = (slot_row[:, :, :, :, None] == rows) & (
        slot_valid[:, :, :, :, None] > 0
    )
    s_all = onehot.astype(np.float32).transpose(0, 1, 3, 2, 4).reshape(
        n_cores, W, P, T * P
    )
    sea_all = (onehot * slot_ea[:, :, :, :, None]).astype(np.float32).transpose(
        0, 1, 3, 2, 4
    ).reshape(n_cores, W, P, T * P)
    earow = slot_ea.reshape(n_cores, W, T * P)

    bin_nodes = np.zeros((NB, P), np.int64)
    bin_nodes[node_bin, node_row] = np.arange(N)
    xw = x.T[:, bin_nodes.reshape(-1)].reshape(P, NB, P)

    xT = x.T
    att_rep2 = np.tile(att.reshape(1, HC), (P, 2))
    we_row = We.reshape(1, HC)
    we_rep = np.broadcast_to(We.reshape(1, HC), (P, HC))
    bias_neg = (-bias).reshape(1, HC)
    ones_r = np.ones((1, P), np.float32)

    in_maps = []
    for c in range(n_cores):
        flat_s = slot_src[c].reshape(-1)
        flat_d = slot_dstn[c].reshape(-1)
        in_maps.append(
            dict(
                xTe=np.ascontiguousarray(xT[:, flat_s].astype(BF)),
                xTr=np.ascontiguousarray(xT[:, flat_d].astype(BF)),
                s_all=np.ascontiguousarray(s_all[c].astype(BF)),
                sea_all=np.ascontiguousarray(sea_all[c].astype(BF)),
                earow=np.ascontiguousarray(earow[c].astype(BF)),
                xw=np.ascontiguousarray(
                    xw[:, c * W : (c + 1) * W, :].transpose(1, 0, 2).astype(BF)
                ),
                Wl=Wl.astype(BF),
                Wr=Wr.astype(BF),
                we_row=np.ascontiguousarray(we_row.astype(BF)),
                we_rep=np.ascontiguousarray(we_rep.astype(np.float32)),
                att_rep2=np.ascontiguousarray(att_rep2.astype(BF)),
                bias_neg=np.ascontiguousarray(bias_neg.astype(BF)),
                ones_r=np.ascontiguousarray(ones_r.astype(BF)),
            )
        )
    meta = dict(
        W=W, T=T, HC=HC, H=H, ND=ND, NDpad=W * P, n_cores=n_cores,
        bin_nodes=bin_nodes, bin_nused=slots_used.copy(),
    )
    return in_maps, meta


def build(meta):
    W, T, HC, H = meta["W"], meta["T"], meta["HC"], meta["H"]
    NDpad = meta["NDpad"]
    WT = W * T

    nc = bacc.Bacc("TRN2", target_bir_lowering=False, debug=False)

    xTe = nc.dram_tensor("xTe", [P, WT * P], BF16, kind="ExternalInput")
    xTr = nc.dram_tensor("xTr", [P, WT * P], BF16, kind="ExternalInput")
    s_all = nc.dram_tensor("s_all", [W, P, T * P], BF16, kind="ExternalInput")
    sea_all = nc.dram_tensor("sea_all", [W, P, T * P], BF16, kind="ExternalInput")
    earow = nc.dram_tensor("earow", [W, T * P], BF16, kind="ExternalInput")
    xw = nc.dram_tensor("xw", [W, P, P], BF16, kind="ExternalInput")
    Wl = nc.dram_tensor("Wl", [P, HC], BF16, kind="ExternalInput")
    Wr = nc.dram_tensor("Wr", [P, HC], BF16, kind="ExternalInput")
    we_row = nc.dram_tensor("we_row", [1, HC], BF16, kind="ExternalInput")
    we_rep = nc.dram_tensor("we_rep", [P, HC], F32, kind="ExternalInput")
    att_rep2 = nc.dram_tensor("att_rep2", [P, 2 * HC], BF16, kind="ExternalInput")
    bias_neg = nc.dram_tensor("bias_neg", [1, HC], BF16, kind="ExternalInput")
    ones_r = nc.dram_tensor("ones_r", [1, P], BF16, kind="ExternalInput")
    out = nc.dram_tensor("out", [NDpad, HC], F32, kind="ExternalOutput")

    n_pair = T // 2
    n_tail = T - 2 * n_pair
    ppw = n_pair + n_tail  # pair-units per window
    units = []
    for w in range(W):
        for pi in range(ppw):
            t0 = 2 * pi
            cnt = 2 if pi < n_pair else 1
            units.append((w, t0, cnt))
    U = len(units)
    DEPTH = 4  # scatter trails z by this many units

    with tile.TileContext(nc) as tc, ExitStack() as ctx:
        cpool = ctx.enter_context(tc.tile_pool(name="cpool", bufs=1))
        wl_t = cpool.tile([P, HC], BF16)
        nc.sync.dma_start(wl_t[:], Wl[:, :])
        wr_t = cpool.tile([P, HC], BF16)
        nc.sync.dma_start(wr_t[:], Wr[:, :])
        werow_t = cpool.tile([1, HC], BF16)
        nc.sync.dma_start(werow_t[:], we_row[:, :])
        werep_t = cpool.tile([P, HC], F32)
        nc.sync.dma_start(werep_t[:], we_rep[:, :])
        attrep2_t = cpool.tile([P, 2 * HC], BF16)
        nc.sync.dma_start(attrep2_t[:], att_rep2[:, :])
        biasneg_row = cpool.tile([1, HC], BF16)
        nc.sync.dma_start(biasneg_row[:], bias_neg[:, :])
        ones_row = cpool.tile([1, P], BF16)
        nc.sync.dma_start(ones_row[:], ones_r[:, :])

        with tc.tile_pool(name="win", bufs=2) as winp, tc.tile_pool(
            name="sub", bufs=6
        ) as subp, tc.tile_pool(name="xsps", bufs=5, space="PSUM") as xsps, tc.tile_pool(
            name="aggps", bufs=1, space="PSUM"
        ) as aggps, tc.tile_pool(name="denps", bufs=1, space="PSUM") as denps, tc.tile_pool(
            name="aeaps", bufs=1, space="PSUM"
        ) as aeaps, tc.tile_pool(name="post", bufs=2) as postp:
            wstate = {}  # w -> dict of window tiles
            ustate = {}  # i -> dict of per-unit tiles

            def open_window(w):
                st = {}
                st["xe"] = winp.tile([P, T * P], BF16, tag="xe", name="xe_t")
                nc.sync.dma_start(st["xe"][:], xTe[:, w * T * P : (w + 1) * T * P])
                st["xr"] = winp.tile([P, T * P], BF16, tag="xr", name="xr_t")
                nc.sync.dma_start(st["xr"][:], xTr[:, w * T * P : (w + 1) * T * P])
                st["S"] = winp.tile([P, T * P], BF16, tag="S", name="S_t")
                nc.sync.dma_start(st["S"][:], s_all[w, :, :])
                st["Sea"] = winp.tile([P, T * P], BF16, tag="Sea", name="Sea_t")
                nc.sync.dma_start(st["Sea"][:], sea_all[w, :, :])
                st["ear"] = winp.tile([1, T * P], BF16, tag="ear", name="ear_t")
                nc.sync.dma_start(st["ear"][:], earow[w : w + 1, :])
                st["xw"] = winp.tile([P, P], BF16, tag="xw", name="xw_t")
                nc.sync.dma_start(st["xw"][:], xw[w, :, :])
                st["agg"] = aggps.tile([P, 2 * HC], F32, tag="agg", name="agg")
                st["den"] = denps.tile([P, H], F32, tag="den", name="den")
                st["aea"] = aeaps.tile([P, H], F32, tag="aea", name="aea")
                return st

            def stage_z(i):
                w, t0, cnt = units[i]
                if w not in wstate:
                    wstate[w] = open_window(w)
                st = wstate[w]
                xs = xsps.tile([P, 2 * HC], F32, tag="xs")
                for k in range(cnt):
                    t = t0 + k
                    reg = xs[:, k * HC : (k + 1) * HC]
                    xe_t = st["xe"][:, t * P : (t + 1) * P]
                    xr_t = st["xr"][:, t * P : (t + 1) * P]
                    nc.tensor.matmul(reg, xe_t, wl_t[:], start=True, stop=False)
                    nc.tensor.matmul(reg, xr_t, wr_t[:], start=False, stop=False)
                    nc.tensor.matmul(
                        reg, st["ear"][0:1, t * P : (t + 1) * P], werow_t[:],
                        start=False, stop=True,
                    )
                ustate[i] = dict(xs=xs)

            def stage_Ltm(i):
                w, t0, cnt = units[i]
                u = ustate[i]
                L2 = subp.tile([P, 2 * HC], BF16, tag="L2")
                nc.scalar.activation(
                    out=L2[:, 0 : cnt * HC], in_=u["xs"][:, 0 : cnt * HC],
                    func=mybir.ActivationFunctionType.Prelu,
                    bias=0.0, scale=1.0, alpha=NEG,
                )
                tm2 = subp.tile([P, 2 * HC], BF16, tag="tm2")
                nc.gpsimd.tensor_tensor(
                    out=tm2[:, 0 : cnt * HC], in0=L2[:, 0 : cnt * HC],
                    in1=attrep2_t[:, 0 : cnt * HC], op=mybir.AluOpType.mult,
                )
                u["tm2"] = tm2

            def stage_redexp(i):
                w, t0, cnt = units[i]
                u = ustate[i]
                lg = subp.tile([P, 2 * H], F32, tag="lg")
                nc.vector.tensor_reduce(
                    out=lg[:, 0 : cnt * H],
                    in_=u["tm2"][:, 0 : cnt * HC].rearrange(
                        "p (u h c) -> p u h c", u=cnt, c=32
                    ),
                    axis=mybir.AxisListType.X,
                    op=mybir.AluOpType.add,
                )
                ext = subp.tile([P, 2 * H], BF16, tag="ext")
                nc.scalar.activation(
                    out=ext[:, 0 : cnt * H], in_=lg[:, 0 : cnt * H],
                    func=mybir.ActivationFunctionType.Exp,
                )
                u["ext"] = ext

            def stage_scatter(i):
                w, t0, cnt = units[i]
                u = ustate[i]
                st = wstate[w]
                exy = subp.tile([P, 2 * HC], BF16, tag="exy")
                nc.vector.tensor_tensor(
                    out=exy[:, 0 : cnt * HC].rearrange(
                        "p (u h c) -> p u h c", u=cnt, c=32
                    ),
                    in0=u["xs"][:, 0 : cnt * HC].rearrange(
                        "p (u h c) -> p u h c", u=cnt, c=32
                    ),
                    in1=u["ext"][:, 0 : cnt * H]
                    .rearrange("p (u h) -> p u h", u=cnt)
                    .unsqueeze(3)
                    .to_broadcast([P, cnt, H, 32]),
                    op=mybir.AluOpType.mult,
                )
                for k in range(cnt):
                    t = t0 + k
                    S_t = st["S"][:, t * P : (t + 1) * P]
                    nc.tensor.matmul(
                        st["agg"][:, 0:HC], S_t, exy[:, k * HC : (k + 1) * HC],
                        start=(t == 0), stop=(t == T - 1),
                    )
                    nc.tensor.matmul(
                        st["den"][:], S_t,
                        u["ext"][:, k * H : (k + 1) * H],
                        start=(t == 0), stop=(t == T - 1),
                    )
                    nc.tensor.matmul(
                        st["aea"][:],
                        st["Sea"][:, t * P : (t + 1) * P],
                        u["ext"][:, k * H : (k + 1) * H],
                        start=(t == 0), stop=(t == T - 1),
                    )
                del u["xs"]

            fstate = {}

            def finish_window_A(w):
                st = wstate[w]
                xrw = st["agg"][:, HC : 2 * HC]
                nc.tensor.matmul(
                    xrw, st["xw"][:], wr_t[:], start=True, stop=False
                )
                nc.tensor.matmul(
                    xrw, ones_row[:], biasneg_row[:], start=False, stop=True
                )
                rc = postp.tile([P, H], F32, tag="rc")
                nc.vector.reciprocal(rc[:], st["den"][:])
                sh = postp.tile([P, H], F32, tag="sh")
                nc.vector.tensor_tensor(
                    out=sh[:], in0=st["aea"][:], in1=rc[:],
                    op=mybir.AluOpType.mult,
                )
                G = postp.tile([P, HC], F32, tag="G")
                nc.vector.tensor_tensor(
                    out=G[:].rearrange("p (h c) -> p h c", c=32),
                    in0=st["agg"][:, 0:HC].rearrange("p (h c) -> p h c", c=32),
                    in1=rc[:].unsqueeze(2).to_broadcast([P, H, 32]),
                    op=mybir.AluOpType.mult,
                )
                Hh = postp.tile([P, HC], F32, tag="Hh")
                nc.gpsimd.tensor_tensor(
                    out=Hh[:].rearrange("p (h c) -> p h c", c=32),
                    in0=werep_t[:].rearrange("p (h c) -> p h c", c=32),
                    in1=sh[:].unsqueeze(2).to_broadcast([P, H, 32]),
                    op=mybir.AluOpType.mult,
                )
                fstate[w] = dict(G=G, Hh=Hh)

            def finish_window_B(w):
                st = wstate.pop(w)
                fs = fstate.pop(w)
                xrw = st["agg"][:, HC : 2 * HC]
                I2 = postp.tile([P, HC], F32, tag="I2")
                nc.gpsimd.tensor_tensor(
                    out=I2[:], in0=fs["G"][:], in1=fs["Hh"][:],
                    op=mybir.AluOpType.subtract,
                )
                out1 = postp.tile([P, HC], F32, tag="out1")
                nc.vector.scalar_tensor_tensor(
                    out=out1[:], in0=xrw, scalar=-1.0, in1=I2[:],
                    op0=mybir.AluOpType.mult, op1=mybir.AluOpType.add,
                )
                nc.sync.dma_start(out[w * P : (w + 1) * P, :], out1[:])

            for i in range(U + DEPTH):
                if i < U:
                    stage_z(i)
                j = i - DEPTH
                if j >= 0:
                    stage_scatter(j)
                if i - 1 >= 0 and i - 1 < U:
                    stage_Ltm(i - 1)
                if i - 2 >= 0 and i - 2 < U:
                    stage_redexp(i - 2)
                if j >= 0:
                    if j == U - 1 or units[j + 1][0] != units[j][0]:
                        finish_window_A(units[j][0])
                    if j - 1 >= 0 and (
                        j - 1 == U - 1 or units[j][0] != units[j - 1][0]
                    ):
                        finish_window_B(units[j - 1][0])
                    ustate.pop(j, None)
            if U > 0:
                finish_window_B(units[U - 1][0])

    nc.compile()
    return nc


_LAST_RESULT = None


def kernel(**inputs):
    """Full-input GATv2 forward on 8 TRN2 NeuronCores (dst-sharded)."""
    global _LAST_RESULT
    n_cores = 8
    in_maps, meta = preprocess(
        inputs["x"], inputs["edge_index"], inputs["edge_attr"],
        inputs["Wl"], inputs["Wr"], inputs["We"], inputs["att"],
        inputs["bias"], n_cores,
    )
    nc = build(meta)
    from concourse.bass_utils import run_bass_kernel_spmd

    res = run_bass_kernel_spmd(nc, in_maps, core_ids=list(range(n_cores)))
    _LAST_RESULT = res
    W = meta["W"]
    N = meta["ND"] * n_cores
    HC = meta["HC"]
    bin_nodes = meta["bin_nodes"]
    bin_nused = meta["bin_nused"]
    full = np.concatenate(
        [np.asarray(res.results[c]["out"]).reshape(W * P, HC) for c in range(n_cores)],
        axis=0,
    ).reshape(-1, P, HC)
    out = np.zeros((N, HC), np.float32)
    real = np.arange(P)[None, :] < bin_nused[:, None]
    out[bin_nodes[real]] = full[real]
    return np.ascontiguousarray(out.astype(np.float32))


# revision 3
# speedup vs baseline: 1.4482x; 1.0680x over previous
"""GATv2 kernel v4: software-pipelined pair stream.

Same math/staging as v3 (merged self-loops, LPT-balanced bins, bf16 tables,
sum(alpha)=1 correction) but restructured as one flat stream of subtile
pairs with stage offsets so every engine stays busy:
  step i:  PE z(i) | DVE exY(i-3) | PE scatter(i-3) | Act lrelu(i-1)
           | Pool att-mult(i-1) | DVE head-reduce(i-2) | Act exp(i-2)
Padding slots are masked by zeros in the S/Sea scatter tables (no vmask).
"""

import numpy as np
from contextlib import ExitStack

import concourse.bass as bass
import concourse.tile as tile
from concourse import bacc, mybir

F32 = mybir.dt.float32
BF16 = mybir.dt.bfloat16
P = 128
NEG = 0.2


def preprocess(x, edge_index, edge_attr, Wl, Wr, We, att, bias, n_cores):
    import ml_dtypes

    BF = ml_dtypes.bfloat16
    x = np.ascontiguousarray(np.asarray(x, np.float32))
    src = np.asarray(edge_index[0]).astype(np.int64)
    dst = np.asarray(edge_index[1]).astype(np.int64)
    ea = np.asarray(edge_attr, np.float32).reshape(-1)
    Wl = np.ascontiguousarray(np.asarray(Wl, np.float32))
    Wr = np.ascontiguousarray(np.asarray(Wr, np.float32))
    We = np.asarray(We, np.float32).reshape(-1)
    att = np.asarray(att, np.float32)
    bias = np.asarray(bias, np.float32).reshape(-1)

    N, F = x.shape
    HC = Wl.shape[1]
    H = HC // 32
    assert F == P
    ND = N // n_cores
    W = (ND + P - 1) // P
    NB = n_cores * W

    cnt = np.bincount(dst, minlength=N).astype(np.int64)
    sums = np.zeros(N, np.float64)
    np.add.at(sums, dst, ea)
    la = (sums / np.maximum(cnt, 1)).astype(np.float32)

    src_f = np.concatenate([src, np.arange(N, dtype=np.int64)])
    dst_f = np.concatenate([dst, np.arange(N, dtype=np.int64)])
    ea_f = np.concatenate([ea, la])
    deg = cnt + 1

    import heapq

    order = np.argsort(-deg, kind="stable")
    heap = [(0, b) for b in range(NB)]
    heapq.heapify(heap)
    slots_used = np.zeros(NB, np.int64)
    node_bin = np.zeros(N, np.int64)
    node_row = np.zeros(N, np.int64)
    stash = []
    for n in order:
        while True:
            load, b = heapq.heappop(heap)
            if slots_used[b] < P:
                break
            stash.append((load, b))
        node_bin[n] = b
        node_row[n] = slots_used[b]
        slots_used[b] += 1
        heapq.heappush(heap, (load + int(deg[n]), b))
        for it in stash:
            heapq.heappush(heap, it)
        stash.clear()

    binload = np.bincount(node_bin[dst_f], minlength=NB)
    T = int(np.ceil(binload.max() / P))

    ebin = node_bin[dst_f]
    eorder = np.argsort(ebin, kind="stable")
    src_s = src_f[eorder]
    dst_s = dst_f[eorder]
    ea_s = ea_f[eorder]
    ebin_s = ebin[eorder]
    starts = np.zeros(NB, np.int64)
    np.cumsum(binload[:-1], out=starts[1:])
    rank = np.arange(len(src_s)) - starts[ebin_s]
    t_of = rank // P
    p_of = rank % P

    core = ebin_s // W
    w_of = ebin_s % W

    slot_src = np.zeros((n_cores, W, T, P), np.int64)
    slot_dstn = np.zeros((n_cores, W, T, P), np.int64)
    slot_row = np.zeros((n_cores, W, T, P), np.int64)
    slot_ea = np.zeros((n_cores, W, T, P), np.float32)
    slot_valid = np.zeros((n_cores, W, T, P), np.float32)

    slot_src[core, w_of, t_of, p_of] = src_s
    slot_dstn[core, w_of, t_of, p_of] = dst_s
    slot_row[core, w_of, t_of, p_of] = node_row[dst_s]
    slot_ea[core, w_of, t_of, p_of] = ea_s
    slot_valid[core, w_of, t_of, p_of] = 1.0

    rows = np.arange(P).reshape(1, 1, 1, 1, P)
    onehot = (slot_row[:, :, :, :, None] == rows) & (
        slot_valid[:, :, :, :, None] > 0
    )
    s_all = onehot.astype(np.float32).transpose(0, 1, 3, 2, 4).reshape(
        n_cores, W, P, T * P
    )
    sea_all = (onehot * slot_ea[:, :, :, :, None]).astype(np.float32).transpose(
        0, 1, 3, 2, 4
    ).reshape(n_cores, W, P, T * P)
    earow = slot_ea.reshape(n_cores, W, T * P)

    bin_nodes = np.zeros((NB, P), np.int64)
    bin_nodes[node_bin, node_row] = np.arange(N)
    xw = x.T[:, bin_nodes.reshape(-1)].reshape(P, NB, P)

    xT = x.T
    att_rep2 = np.tile(att.reshape(1, HC), (P, 2))
    we_row = We.reshape(1, HC)
    we_rep = np.broadcast_to(We.reshape(1, HC), (P, HC))
    bias_neg = (-bias).reshape(1, HC)
    ones_r = np.ones((1, P), np.float32)

    in_maps = []
    for c in range(n_cores):
        flat_s = slot_src[c].reshape(-1)
        flat_d = slot_dstn[c].reshape(-1)
        in_maps.append(
            dict(
                xTe=np.ascontiguousarray(xT[:, flat_s].astype(BF)),
                xTr=np.ascontiguousarray(xT[:, flat_d].astype(BF)),
                s_all=np.ascontiguousarray(s_all[c].astype(BF)),
                sea_all=np.ascontiguousarray(sea_all[c].astype(BF)),
                earow=np.ascontiguousarray(earow[c].astype(BF)),
                xw=np.ascontiguousarray(
                    xw[:, c * W : (c + 1) * W, :].transpose(1, 0, 2).astype(BF)
                ),
                Wl=Wl.astype(BF),
                Wr=Wr.astype(BF),
                we_row=np.ascontiguousarray(we_row.astype(BF)),
                we_rep=np.ascontiguousarray(we_rep.astype(np.float32)),
                att_rep2=np.ascontiguousarray(att_rep2.astype(BF)),
                bias_neg=np.ascontiguousarray(bias_neg.astype(BF)),
                ones_r=np.ascontiguousarray(ones_r.astype(BF)),
            )
        )
    meta = dict(
        W=W, T=T, HC=HC, H=H, ND=ND, NDpad=W * P, n_cores=n_cores,
        bin_nodes=bin_nodes, bin_nused=slots_used.copy(),
    )
    return in_maps, meta


def build(meta):
    W, T, HC, H = meta["W"], meta["T"], meta["HC"], meta["H"]
    NDpad = meta["NDpad"]
    WT = W * T

    nc = bacc.Bacc("TRN2", target_bir_lowering=False, debug=False)

    xTe = nc.dram_tensor("xTe", [P, WT * P], BF16, kind="ExternalInput")
    xTr = nc.dram_tensor("xTr", [P, WT * P], BF16, kind="ExternalInput")
    s_all = nc.dram_tensor("s_all", [W, P, T * P], BF16, kind="ExternalInput")
    sea_all = nc.dram_tensor("sea_all", [W, P, T * P], BF16, kind="ExternalInput")
    earow = nc.dram_tensor("earow", [W, T * P], BF16, kind="ExternalInput")
    xw = nc.dram_tensor("xw", [W, P, P], BF16, kind="ExternalInput")
    Wl = nc.dram_tensor("Wl", [P, HC], BF16, kind="ExternalInput")
    Wr = nc.dram_tensor("Wr", [P, HC], BF16, kind="ExternalInput")
    we_row = nc.dram_tensor("we_row", [1, HC], BF16, kind="ExternalInput")
    we_rep = nc.dram_tensor("we_rep", [P, HC], F32, kind="ExternalInput")
    att_rep2 = nc.dram_tensor("att_rep2", [P, 2 * HC], BF16, kind="ExternalInput")
    bias_neg = nc.dram_tensor("bias_neg", [1, HC], BF16, kind="ExternalInput")
    ones_r = nc.dram_tensor("ones_r", [1, P], BF16, kind="ExternalInput")
    out = nc.dram_tensor("out", [NDpad, HC], F32, kind="ExternalOutput")

    n_pair = T // 2
    n_tail = T - 2 * n_pair
    ppw = n_pair + n_tail  # pair-units per window
    units = []
    for w in range(W):
        for pi in range(ppw):
            t0 = 2 * pi
            cnt = 2 if pi < n_pair else 1
            units.append((w, t0, cnt))
    U = len(units)
    DEPTH = 4  # scatter trails z by this many units

    with tile.TileContext(nc) as tc, ExitStack() as ctx:
        cpool = ctx.enter_context(tc.tile_pool(name="cpool", bufs=1))
        wl_t = cpool.tile([P, HC], BF16)
        nc.sync.dma_start(wl_t[:], Wl[:, :])
        wr_t = cpool.tile([P, HC], BF16)
        nc.sync.dma_start(wr_t[:], Wr[:, :])
        werow_t = cpool.tile([1, HC], BF16)
        nc.sync.dma_start(werow_t[:], we_row[:, :])
        werep_t = cpool.tile([P, HC], F32)
        nc.sync.dma_start(werep_t[:], we_rep[:, :])
        attrep2_t = cpool.tile([P, 2 * HC], BF16)
        nc.sync.dma_start(attrep2_t[:], att_rep2[:, :])
        biasneg_row = cpool.tile([1, HC], BF16)
        nc.sync.dma_start(biasneg_row[:], bias_neg[:, :])
        ones_row = cpool.tile([1, P], BF16)
        nc.sync.dma_start(ones_row[:], ones_r[:, :])

        with tc.tile_pool(name="win", bufs=2) as winp, tc.tile_pool(
            name="sub", bufs=6
        ) as subp, tc.tile_pool(name="xsps", bufs=5, space="PSUM") as xsps, tc.tile_pool(
            name="aggps", bufs=1, space="PSUM"
        ) as aggps, tc.tile_pool(name="denps", bufs=1, space="PSUM") as denps, tc.tile_pool(
            name="aeaps", bufs=1, space="PSUM"
        ) as aeaps, tc.tile_pool(name="post", bufs=2) as postp:
            wstate = {}  # w -> dict of window tiles
            ustate = {}  # i -> dict of per-unit tiles

            def open_window(w):
                st = {}
                st["xe"] = winp.tile([P, T * P], BF16, tag="xe", name="xe_t")
                nc.sync.dma_start(st["xe"][:], xTe[:, w * T * P : (w + 1) * T * P])
                st["xr"] = winp.tile([P, T * P], BF16, tag="xr", name="xr_t")
                nc.sync.dma_start(st["xr"][:], xTr[:, w * T * P : (w + 1) * T * P])
                st["ear"] = winp.tile([1, T * P], BF16, tag="ear", name="ear_t")
                nc.sync.dma_start(st["ear"][:], earow[w : w + 1, :])
                st["S"] = winp.tile([P, T * P], BF16, tag="S", name="S_t")
                nc.sync.dma_start(st["S"][:], s_all[w, :, :])
                st["Sea"] = winp.tile([P, T * P], BF16, tag="Sea", name="Sea_t")
                nc.sync.dma_start(st["Sea"][:], sea_all[w, :, :])
                st["xw"] = winp.tile([P, P], BF16, tag="xw", name="xw_t")
                nc.sync.dma_start(st["xw"][:], xw[w, :, :])
                st["agg"] = aggps.tile([P, 2 * HC], F32, tag="agg", name="agg")
                st["den"] = denps.tile([P, H], F32, tag="den", name="den")
                st["aea"] = aeaps.tile([P, H], F32, tag="aea", name="aea")
                return st

            def stage_z(i):
                w, t0, cnt = units[i]
                if w not in wstate:
                    wstate[w] = open_window(w)
                st = wstate[w]
                xs = xsps.tile([P, 2 * HC], F32, tag="xs")
                for k in range(cnt):
                    t = t0 + k
                    reg = xs[:, k * HC : (k + 1) * HC]
                    xe_t = st["xe"][:, t * P : (t + 1) * P]
                    xr_t = st["xr"][:, t * P : (t + 1) * P]
                    nc.tensor.matmul(reg, xe_t, wl_t[:], start=True, stop=False)
                    nc.tensor.matmul(reg, xr_t, wr_t[:], start=False, stop=False)
                    nc.tensor.matmul(
                        reg, st["ear"][0:1, t * P : (t + 1) * P], werow_t[:],
                        start=False, stop=True,
                    )
                ustate[i] = dict(xs=xs)

            def stage_Ltm(i):
                w, t0, cnt = units[i]
                u = ustate[i]
                u["presplit"] = False
                L2 = subp.tile([P, 2 * HC], BF16, tag="L2")
                nc.scalar.activation(
                    out=L2[:, 0 : cnt * HC], in_=u["xs"][:, 0 : cnt * HC],
                    func=mybir.ActivationFunctionType.Prelu,
                    bias=0.0, scale=1.0, alpha=NEG,
                )
                tm2 = subp.tile([P, 2 * HC], BF16, tag="tm2")
                nc.gpsimd.tensor_tensor(
                    out=tm2[:, 0 : cnt * HC], in0=L2[:, 0 : cnt * HC],
                    in1=attrep2_t[:, 0 : cnt * HC], op=mybir.AluOpType.mult,
                )
                u["tm2"] = tm2
                if u["presplit"]:
                    tmv = tm2[:, 0 : cnt * HC].rearrange(
                        "p (u h c) -> p u h c", u=cnt, c=32
                    )
                    pm = subp.tile([P, HC], BF16, tag="pm")
                    nc.gpsimd.tensor_tensor(
                        out=pm[:, 0 : cnt * HC // 2].rearrange(
                            "p (u h c) -> p u h c", u=cnt, c=16
                        ),
                        in0=tmv[:, :, :, 0:16],
                        in1=tmv[:, :, :, 16:32],
                        op=mybir.AluOpType.add,
                    )
                    u["pm"] = pm

            def stage_redexp(i):
                w, t0, cnt = units[i]
                u = ustate[i]
                lg = subp.tile([P, 2 * H], F32, tag="lg")
                if u["presplit"]:
                    red_in = u["pm"][:, 0 : cnt * HC // 2].rearrange(
                        "p (u h c) -> p u h c", u=cnt, c=16
                    )
                else:
                    red_in = u["tm2"][:, 0 : cnt * HC].rearrange(
                        "p (u h c) -> p u h c", u=cnt, c=32
                    )
                nc.vector.tensor_reduce(
                    out=lg[:, 0 : cnt * H],
                    in_=red_in,
                    axis=mybir.AxisListType.X,
                    op=mybir.AluOpType.add,
                )
                ext = subp.tile([P, 2 * H], BF16, tag="ext")
                nc.scalar.activation(
                    out=ext[:, 0 : cnt * H], in_=lg[:, 0 : cnt * H],
                    func=mybir.ActivationFunctionType.Exp,
                )
                u["ext"] = ext

            def stage_scatter(i):
                w, t0, cnt = units[i]
                u = ustate[i]
                st = wstate[w]
                exy = subp.tile([P, 2 * HC], BF16, tag="exy")
                nc.vector.tensor_tensor(
                    out=exy[:, 0 : cnt * HC].rearrange(
                        "p (u h c) -> p u h c", u=cnt, c=32
                    ),
                    in0=u["xs"][:, 0 : cnt * HC].rearrange(
                        "p (u h c) -> p u h c", u=cnt, c=32
                    ),
                    in1=u["ext"][:, 0 : cnt * H]
                    .rearrange("p (u h) -> p u h", u=cnt)
                    .unsqueeze(3)
                    .to_broadcast([P, cnt, H, 32]),
                    op=mybir.AluOpType.mult,
                )
                for k in range(cnt):
                    t = t0 + k
                    S_t = st["S"][:, t * P : (t + 1) * P]
                    nc.tensor.matmul(
                        st["agg"][:, 0:HC], S_t, exy[:, k * HC : (k + 1) * HC],
                        start=(t == 0), stop=(t == T - 1),
                    )
                    nc.tensor.matmul(
                        st["den"][:], S_t,
                        u["ext"][:, k * H : (k + 1) * H],
                        start=(t == 0), stop=(t == T - 1),
                    )
                    nc.tensor.matmul(
                        st["aea"][:],
                        st["Sea"][:, t * P : (t + 1) * P],
                        u["ext"][:, k * H : (k + 1) * H],
                        start=(t == 0), stop=(t == T - 1),
                    )
                del u["xs"]

            fstate = {}

            def finish_window_A(w):
                st = wstate[w]
                xrw = st["agg"][:, HC : 2 * HC]
                nc.tensor.matmul(
                    xrw, st["xw"][:], wr_t[:], start=True, stop=False
                )
                nc.tensor.matmul(
                    xrw, ones_row[:], biasneg_row[:], start=False, stop=True
                )
                rc = postp.tile([P, H], F32, tag="rc")
                nc.vector.reciprocal(rc[:], st["den"][:])
                sh = postp.tile([P, H], F32, tag="sh")
                nc.vector.tensor_tensor(
                    out=sh[:], in0=st["aea"][:], in1=rc[:],
                    op=mybir.AluOpType.mult,
                )
                G = postp.tile([P, HC], F32, tag="G")
                nc.vector.tensor_tensor(
                    out=G[:].rearrange("p (h c) -> p h c", c=32),
                    in0=st["agg"][:, 0:HC].rearrange("p (h c) -> p h c", c=32),
                    in1=rc[:].unsqueeze(2).to_broadcast([P, H, 32]),
                    op=mybir.AluOpType.mult,
                )
                Hh = postp.tile([P, HC], F32, tag="Hh")
                nc.vector.tensor_tensor(
                    out=Hh[:].rearrange("p (h c) -> p h c", c=32),
                    in0=werep_t[:].rearrange("p (h c) -> p h c", c=32),
                    in1=sh[:].unsqueeze(2).to_broadcast([P, H, 32]),
                    op=mybir.AluOpType.mult,
                )
                fstate[w] = dict(G=G, Hh=Hh)

            def finish_window_B1(w):
                fs = fstate[w]
                I2 = postp.tile([P, HC], F32, tag="I2")
                nc.vector.scalar_tensor_tensor(
                    out=I2[:], in0=fs["Hh"][:], scalar=-1.0, in1=fs["G"][:],
                    op0=mybir.AluOpType.mult, op1=mybir.AluOpType.add,
                )
                fs["I2"] = I2

            def finish_window_B2(w):
                st = wstate.pop(w)
                fs = fstate.pop(w)
                xrw = st["agg"][:, HC : 2 * HC]
                out1 = postp.tile([P, HC], F32, tag="out1")
                nc.vector.scalar_tensor_tensor(
                    out=out1[:], in0=xrw, scalar=-1.0, in1=fs["I2"][:],
                    op0=mybir.AluOpType.mult, op1=mybir.AluOpType.add,
                )
                nc.sync.dma_start(out[w * P : (w + 1) * P, :], out1[:])

            for i in range(U + DEPTH):
                if i < U:
                    stage_z(i)
                j = i - DEPTH
                if j >= 0:
                    stage_scatter(j)
                    if 0 <= j - 1 < U and (
                        j - 1 == U - 1 or units[j][0] != units[j - 1][0]
                    ):
                        finish_window_B1(units[j - 1][0])
                if i - 1 >= 0 and i - 1 < U:
                    stage_Ltm(i - 1)
                if i - 2 >= 0 and i - 2 < U:
                    stage_redexp(i - 2)
                def is_last(k):
                    return 0 <= k < U and (
                        k == U - 1 or units[k + 1][0] != units[k][0]
                    )

                if j >= 0:
                    if is_last(j):
                        finish_window_A(units[j][0])
                    if is_last(j - 2):
                        finish_window_B2(units[j - 2][0])
                    ustate.pop(j, None)
            if U > 0:
                wl_last = units[U - 1][0]
                if wl_last in fstate and "I2" not in fstate[wl_last]:
                    finish_window_B1(wl_last)
                if wl_last in fstate:
                    finish_window_B2(wl_last)

    nc.compile()
    return nc


_LAST_RESULT = None


def kernel(**inputs):
    """Full-input GATv2 forward on 8 TRN2 NeuronCores (dst-sharded)."""
    global _LAST_RESULT
    n_cores = 8
    in_maps, meta = preprocess(
        inputs["x"], inputs["edge_index"], inputs["edge_attr"],
        inputs["Wl"], inputs["Wr"], inputs["We"], inputs["att"],
        inputs["bias"], n_cores,
    )
    nc = build(meta)
    from concourse.bass_utils import run_bass_kernel_spmd

    res = run_bass_kernel_spmd(nc, in_maps, core_ids=list(range(n_cores)))
    _LAST_RESULT = res
    W = meta["W"]
    N = meta["ND"] * n_cores
    HC = meta["HC"]
    bin_nodes = meta["bin_nodes"]
    bin_nused = meta["bin_nused"]
    full = np.concatenate(
        [np.asarray(res.results[c]["out"]).reshape(W * P, HC) for c in range(n_cores)],
        axis=0,
    ).reshape(-1, P, HC)
    out = np.zeros((N, HC), np.float32)
    real = np.arange(P)[None, :] < bin_nused[:, None]
    out[bin_nodes[real]] = full[real]
    return np.ascontiguousarray(out.astype(np.float32))


# revision 4
# speedup vs baseline: 1.4588x; 1.0073x over previous
"""GATv2 kernel v4: software-pipelined pair stream.

Same math/staging as v3 (merged self-loops, LPT-balanced bins, bf16 tables,
sum(alpha)=1 correction) but restructured as one flat stream of subtile
pairs with stage offsets so every engine stays busy:
  step i:  PE z(i) | DVE exY(i-3) | PE scatter(i-3) | Act lrelu(i-1)
           | Pool att-mult(i-1) | DVE head-reduce(i-2) | Act exp(i-2)
Padding slots are masked by zeros in the S/Sea scatter tables (no vmask).
"""

import numpy as np
from contextlib import ExitStack

import concourse.bass as bass
import concourse.tile as tile
from concourse import bacc, mybir

F32 = mybir.dt.float32
BF16 = mybir.dt.bfloat16
P = 128
NEG = 0.2


def preprocess(x, edge_index, edge_attr, Wl, Wr, We, att, bias, n_cores):
    import ml_dtypes

    BF = ml_dtypes.bfloat16
    x = np.ascontiguousarray(np.asarray(x, np.float32))
    src = np.asarray(edge_index[0]).astype(np.int64)
    dst = np.asarray(edge_index[1]).astype(np.int64)
    ea = np.asarray(edge_attr, np.float32).reshape(-1)
    Wl = np.ascontiguousarray(np.asarray(Wl, np.float32))
    Wr = np.ascontiguousarray(np.asarray(Wr, np.float32))
    We = np.asarray(We, np.float32).reshape(-1)
    att = np.asarray(att, np.float32)
    bias = np.asarray(bias, np.float32).reshape(-1)

    N, F = x.shape
    HC = Wl.shape[1]
    H = HC // 32
    assert F == P
    ND = N // n_cores
    W = (ND + P - 1) // P
    NB = n_cores * W

    cnt = np.bincount(dst, minlength=N).astype(np.int64)
    sums = np.zeros(N, np.float64)
    np.add.at(sums, dst, ea)
    la = (sums / np.maximum(cnt, 1)).astype(np.float32)

    src_f = np.concatenate([src, np.arange(N, dtype=np.int64)])
    dst_f = np.concatenate([dst, np.arange(N, dtype=np.int64)])
    ea_f = np.concatenate([ea, la])
    deg = cnt + 1

    import heapq

    order = np.argsort(-deg, kind="stable")
    heap = [(0, b) for b in range(NB)]
    heapq.heapify(heap)
    slots_used = np.zeros(NB, np.int64)
    node_bin = np.zeros(N, np.int64)
    node_row = np.zeros(N, np.int64)
    stash = []
    for n in order:
        while True:
            load, b = heapq.heappop(heap)
            if slots_used[b] < P:
                break
            stash.append((load, b))
        node_bin[n] = b
        node_row[n] = slots_used[b]
        slots_used[b] += 1
        heapq.heappush(heap, (load + int(deg[n]), b))
        for it in stash:
            heapq.heappush(heap, it)
        stash.clear()

    binload = np.bincount(node_bin[dst_f], minlength=NB)
    T = int(np.ceil(binload.max() / P))

    ebin = node_bin[dst_f]
    eorder = np.argsort(ebin, kind="stable")
    src_s = src_f[eorder]
    dst_s = dst_f[eorder]
    ea_s = ea_f[eorder]
    ebin_s = ebin[eorder]
    starts = np.zeros(NB, np.int64)
    np.cumsum(binload[:-1], out=starts[1:])
    rank = np.arange(len(src_s)) - starts[ebin_s]
    t_of = rank // P
    p_of = rank % P

    core = ebin_s // W
    w_of = ebin_s % W

    slot_src = np.zeros((n_cores, W, T, P), np.int64)
    slot_dstn = np.zeros((n_cores, W, T, P), np.int64)
    slot_row = np.zeros((n_cores, W, T, P), np.int64)
    slot_ea = np.zeros((n_cores, W, T, P), np.float32)
    slot_valid = np.zeros((n_cores, W, T, P), np.float32)

    slot_src[core, w_of, t_of, p_of] = src_s
    slot_dstn[core, w_of, t_of, p_of] = dst_s
    slot_row[core, w_of, t_of, p_of] = node_row[dst_s]
    slot_ea[core, w_of, t_of, p_of] = ea_s
    slot_valid[core, w_of, t_of, p_of] = 1.0

    rows = np.arange(P).reshape(1, 1, 1, 1, P)
    onehot = (slot_row[:, :, :, :, None] == rows) & (
        slot_valid[:, :, :, :, None] > 0
    )
    s_all = onehot.astype(np.float32).transpose(0, 1, 3, 2, 4).reshape(
        n_cores, W, P, T * P
    )
    sea_all = (onehot * slot_ea[:, :, :, :, None]).astype(np.float32).transpose(
        0, 1, 3, 2, 4
    ).reshape(n_cores, W, P, T * P)
    earow = slot_ea.reshape(n_cores, W, T * P)

    bin_nodes = np.zeros((NB, P), np.int64)
    bin_nodes[node_bin, node_row] = np.arange(N)
    xw = x.T[:, bin_nodes.reshape(-1)].reshape(P, NB, P)

    xT = x.T
    att_rep2 = np.tile(att.reshape(1, HC), (P, 2))
    we_row = We.reshape(1, HC)
    we_rep = np.broadcast_to(We.reshape(1, HC), (P, HC))
    bias_neg = (-bias).reshape(1, HC)
    ones_r = np.ones((1, P), np.float32)

    in_maps = []
    for c in range(n_cores):
        flat_s = slot_src[c].reshape(-1)
        flat_d = slot_dstn[c].reshape(-1)
        in_maps.append(
            dict(
                xTe=np.ascontiguousarray(xT[:, flat_s].astype(BF)),
                xTr=np.ascontiguousarray(xT[:, flat_d].astype(BF)),
                s_all=np.ascontiguousarray(s_all[c].astype(BF)),
                sea_all=np.ascontiguousarray(sea_all[c].astype(BF)),
                earow=np.ascontiguousarray(earow[c].astype(BF)),
                xw=np.ascontiguousarray(
                    xw[:, c * W : (c + 1) * W, :].transpose(1, 0, 2).astype(BF)
                ),
                Wl=Wl.astype(BF),
                Wr=Wr.astype(BF),
                we_row=np.ascontiguousarray(we_row.astype(BF)),
                we_rep=np.ascontiguousarray(we_rep.astype(np.float32)),
                att_rep2=np.ascontiguousarray(att_rep2.astype(BF)),
                bias_neg=np.ascontiguousarray(bias_neg.astype(BF)),
                ones_r=np.ascontiguousarray(ones_r.astype(BF)),
            )
        )
    meta = dict(
        W=W, T=T, HC=HC, H=H, ND=ND, NDpad=W * P, n_cores=n_cores,
        bin_nodes=bin_nodes, bin_nused=slots_used.copy(),
    )
    return in_maps, meta


def build(meta):
    W, T, HC, H = meta["W"], meta["T"], meta["HC"], meta["H"]
    NDpad = meta["NDpad"]
    WT = W * T

    nc = bacc.Bacc("TRN2", target_bir_lowering=False, debug=False)

    xTe = nc.dram_tensor("xTe", [P, WT * P], BF16, kind="ExternalInput")
    xTr = nc.dram_tensor("xTr", [P, WT * P], BF16, kind="ExternalInput")
    s_all = nc.dram_tensor("s_all", [W, P, T * P], BF16, kind="ExternalInput")
    sea_all = nc.dram_tensor("sea_all", [W, P, T * P], BF16, kind="ExternalInput")
    earow = nc.dram_tensor("earow", [W, T * P], BF16, kind="ExternalInput")
    xw = nc.dram_tensor("xw", [W, P, P], BF16, kind="ExternalInput")
    Wl = nc.dram_tensor("Wl", [P, HC], BF16, kind="ExternalInput")
    Wr = nc.dram_tensor("Wr", [P, HC], BF16, kind="ExternalInput")
    we_row = nc.dram_tensor("we_row", [1, HC], BF16, kind="ExternalInput")
    we_rep = nc.dram_tensor("we_rep", [P, HC], F32, kind="ExternalInput")
    att_rep2 = nc.dram_tensor("att_rep2", [P, 2 * HC], BF16, kind="ExternalInput")
    bias_neg = nc.dram_tensor("bias_neg", [1, HC], BF16, kind="ExternalInput")
    ones_r = nc.dram_tensor("ones_r", [1, P], BF16, kind="ExternalInput")
    out = nc.dram_tensor("out", [NDpad, HC], F32, kind="ExternalOutput")

    n_pair = T // 2
    n_tail = T - 2 * n_pair
    ppw = n_pair + n_tail  # pair-units per window
    units = []
    for w in range(W):
        for pi in range(ppw):
            t0 = 2 * pi
            cnt = 2 if pi < n_pair else 1
            units.append((w, t0, cnt))
    U = len(units)
    DEPTH = 4  # scatter trails z by this many units

    with tile.TileContext(nc) as tc, ExitStack() as ctx:
        cpool = ctx.enter_context(tc.tile_pool(name="cpool", bufs=1))
        wl_t = cpool.tile([P, HC], BF16)
        nc.scalar.dma_start(wl_t[:], Wl[:, :])
        wr_t = cpool.tile([P, HC], BF16)
        nc.scalar.dma_start(wr_t[:], Wr[:, :])
        werow_t = cpool.tile([1, HC], BF16)
        nc.scalar.dma_start(werow_t[:], we_row[:, :])
        werep_t = cpool.tile([P, HC], F32)
        nc.scalar.dma_start(werep_t[:], we_rep[:, :])
        attrep2_t = cpool.tile([P, 2 * HC], BF16)
        nc.scalar.dma_start(attrep2_t[:], att_rep2[:, :])
        biasneg_row = cpool.tile([1, HC], BF16)
        nc.scalar.dma_start(biasneg_row[:], bias_neg[:, :])
        ones_row = cpool.tile([1, P], BF16)
        nc.scalar.dma_start(ones_row[:], ones_r[:, :])

        with tc.tile_pool(name="win", bufs=2) as winp, tc.tile_pool(
            name="sub", bufs=6
        ) as subp, tc.tile_pool(name="xsps", bufs=5, space="PSUM") as xsps, tc.tile_pool(
            name="aggps", bufs=1, space="PSUM"
        ) as aggps, tc.tile_pool(name="denps", bufs=1, space="PSUM") as denps, tc.tile_pool(
            name="aeaps", bufs=1, space="PSUM"
        ) as aeaps, tc.tile_pool(name="post", bufs=2) as postp:
            wstate = {}  # w -> dict of window tiles
            ustate = {}  # i -> dict of per-unit tiles

            def open_window(w):
                st = {}
                st["xe"] = winp.tile([P, T * P], BF16, tag="xe", name="xe_t")
                nc.sync.dma_start(st["xe"][:], xTe[:, w * T * P : (w + 1) * T * P])
                st["xr"] = winp.tile([P, T * P], BF16, tag="xr", name="xr_t")
                nc.sync.dma_start(st["xr"][:], xTr[:, w * T * P : (w + 1) * T * P])
                st["ear"] = winp.tile([1, T * P], BF16, tag="ear", name="ear_t")
                nc.sync.dma_start(st["ear"][:], earow[w : w + 1, :])
                st["S"] = winp.tile([P, T * P], BF16, tag="S", name="S_t")
                nc.sync.dma_start(st["S"][:], s_all[w, :, :])
                st["Sea"] = winp.tile([P, T * P], BF16, tag="Sea", name="Sea_t")
                nc.sync.dma_start(st["Sea"][:], sea_all[w, :, :])
                st["xw"] = winp.tile([P, P], BF16, tag="xw", name="xw_t")
                nc.sync.dma_start(st["xw"][:], xw[w, :, :])
                st["agg"] = aggps.tile([P, 2 * HC], F32, tag="agg", name="agg")
                st["den"] = denps.tile([P, H], F32, tag="den", name="den")
                st["aea"] = aeaps.tile([P, H], F32, tag="aea", name="aea")
                return st

            def stage_z(i):
                w, t0, cnt = units[i]
                if w not in wstate:
                    wstate[w] = open_window(w)
                st = wstate[w]
                xs = xsps.tile([P, 2 * HC], F32, tag="xs")
                for k in range(cnt):
                    t = t0 + k
                    reg = xs[:, k * HC : (k + 1) * HC]
                    xe_t = st["xe"][:, t * P : (t + 1) * P]
                    xr_t = st["xr"][:, t * P : (t + 1) * P]
                    nc.tensor.matmul(reg, xe_t, wl_t[:], start=True, stop=False)
                    nc.tensor.matmul(reg, xr_t, wr_t[:], start=False, stop=False)
                    nc.tensor.matmul(
                        reg, st["ear"][0:1, t * P : (t + 1) * P], werow_t[:],
                        start=False, stop=True,
                    )
                ustate[i] = dict(xs=xs)

            def stage_Ltm(i):
                w, t0, cnt = units[i]
                u = ustate[i]
                u["presplit"] = False
                L2 = subp.tile([P, 2 * HC], BF16, tag="L2")
                nc.scalar.activation(
                    out=L2[:, 0 : cnt * HC], in_=u["xs"][:, 0 : cnt * HC],
                    func=mybir.ActivationFunctionType.Prelu,
                    bias=0.0, scale=1.0, alpha=NEG,
                )
                tm2 = subp.tile([P, 2 * HC], BF16, tag="tm2")
                nc.gpsimd.tensor_tensor(
                    out=tm2[:, 0 : cnt * HC], in0=L2[:, 0 : cnt * HC],
                    in1=attrep2_t[:, 0 : cnt * HC], op=mybir.AluOpType.mult,
                )
                u["tm2"] = tm2
                if u["presplit"]:
                    tmv = tm2[:, 0 : cnt * HC].rearrange(
                        "p (u h c) -> p u h c", u=cnt, c=32
                    )
                    pm = subp.tile([P, HC], BF16, tag="pm")
                    nc.gpsimd.tensor_tensor(
                        out=pm[:, 0 : cnt * HC // 2].rearrange(
                            "p (u h c) -> p u h c", u=cnt, c=16
                        ),
                        in0=tmv[:, :, :, 0:16],
                        in1=tmv[:, :, :, 16:32],
                        op=mybir.AluOpType.add,
                    )
                    u["pm"] = pm

            def stage_redexp(i):
                w, t0, cnt = units[i]
                u = ustate[i]
                lg = subp.tile([P, 2 * H], F32, tag="lg")
                if u["presplit"]:
                    red_in = u["pm"][:, 0 : cnt * HC // 2].rearrange(
                        "p (u h c) -> p u h c", u=cnt, c=16
                    )
                else:
                    red_in = u["tm2"][:, 0 : cnt * HC].rearrange(
                        "p (u h c) -> p u h c", u=cnt, c=32
                    )
                nc.vector.tensor_reduce(
                    out=lg[:, 0 : cnt * H],
                    in_=red_in,
                    axis=mybir.AxisListType.X,
                    op=mybir.AluOpType.add,
                )
                ext = subp.tile([P, 2 * H], BF16, tag="ext")
                nc.scalar.activation(
                    out=ext[:, 0 : cnt * H], in_=lg[:, 0 : cnt * H],
                    func=mybir.ActivationFunctionType.Exp,
                )
                u["ext"] = ext

            def stage_scatter(i):
                w, t0, cnt = units[i]
                u = ustate[i]
                st = wstate[w]
                exy = subp.tile([P, 2 * HC], BF16, tag="exy")
                nc.vector.tensor_tensor(
                    out=exy[:, 0 : cnt * HC].rearrange(
                        "p (u h c) -> p u h c", u=cnt, c=32
                    ),
                    in0=u["xs"][:, 0 : cnt * HC].rearrange(
                        "p (u h c) -> p u h c", u=cnt, c=32
                    ),
                    in1=u["ext"][:, 0 : cnt * H]
                    .rearrange("p (u h) -> p u h", u=cnt)
                    .unsqueeze(3)
                    .to_broadcast([P, cnt, H, 32]),
                    op=mybir.AluOpType.mult,
                )
                for k in range(cnt):
                    t = t0 + k
                    S_t = st["S"][:, t * P : (t + 1) * P]
                    nc.tensor.matmul(
                        st["agg"][:, 0:HC], S_t, exy[:, k * HC : (k + 1) * HC],
                        start=(t == 0), stop=(t == T - 1),
                    )
                    nc.tensor.matmul(
                        st["den"][:], S_t,
                        u["ext"][:, k * H : (k + 1) * H],
                        start=(t == 0), stop=(t == T - 1),
                    )
                    nc.tensor.matmul(
                        st["aea"][:],
                        st["Sea"][:, t * P : (t + 1) * P],
                        u["ext"][:, k * H : (k + 1) * H],
                        start=(t == 0), stop=(t == T - 1),
                    )
                del u["xs"]

            fstate = {}

            def finish_window_A(w):
                st = wstate[w]
                xrw = st["agg"][:, HC : 2 * HC]
                nc.tensor.matmul(
                    xrw, st["xw"][:], wr_t[:], start=True, stop=False
                )
                nc.tensor.matmul(
                    xrw, ones_row[:], biasneg_row[:], start=False, stop=True
                )
                rc = postp.tile([P, H], F32, tag="rc")
                nc.vector.reciprocal(rc[:], st["den"][:])
                sh = postp.tile([P, H], F32, tag="sh")
                nc.vector.tensor_tensor(
                    out=sh[:], in0=st["aea"][:], in1=rc[:],
                    op=mybir.AluOpType.mult,
                )
                G = postp.tile([P, HC], F32, tag="G")
                nc.vector.tensor_tensor(
                    out=G[:].rearrange("p (h c) -> p h c", c=32),
                    in0=st["agg"][:, 0:HC].rearrange("p (h c) -> p h c", c=32),
                    in1=rc[:].unsqueeze(2).to_broadcast([P, H, 32]),
                    op=mybir.AluOpType.mult,
                )
                Hh = postp.tile([P, HC], F32, tag="Hh")
                nc.vector.tensor_tensor(
                    out=Hh[:].rearrange("p (h c) -> p h c", c=32),
                    in0=werep_t[:].rearrange("p (h c) -> p h c", c=32),
                    in1=sh[:].unsqueeze(2).to_broadcast([P, H, 32]),
                    op=mybir.AluOpType.mult,
                )
                fstate[w] = dict(G=G, Hh=Hh)

            def finish_window_B1(w):
                fs = fstate[w]
                I2 = postp.tile([P, HC], F32, tag="I2")
                nc.vector.scalar_tensor_tensor(
                    out=I2[:], in0=fs["Hh"][:], scalar=-1.0, in1=fs["G"][:],
                    op0=mybir.AluOpType.mult, op1=mybir.AluOpType.add,
                )
                fs["I2"] = I2

            def finish_window_B2(w):
                st = wstate.pop(w)
                fs = fstate.pop(w)
                xrw = st["agg"][:, HC : 2 * HC]
                out1 = postp.tile([P, HC], F32, tag="out1")
                nc.vector.scalar_tensor_tensor(
                    out=out1[:], in0=xrw, scalar=-1.0, in1=fs["I2"][:],
                    op0=mybir.AluOpType.mult, op1=mybir.AluOpType.add,
                )
                nc.sync.dma_start(out[w * P : (w + 1) * P, :], out1[:])

            for i in range(U + DEPTH):
                if i < U:
                    stage_z(i)
                j = i - DEPTH
                if j >= 0:
                    stage_scatter(j)
                    if 0 <= j - 1 < U and (
                        j - 1 == U - 1 or units[j][0] != units[j - 1][0]
                    ):
                        finish_window_B1(units[j - 1][0])
                if i - 1 >= 0 and i - 1 < U:
                    stage_Ltm(i - 1)
                if i - 2 >= 0 and i - 2 < U:
                    stage_redexp(i - 2)
                def is_last(k):
                    return 0 <= k < U and (
                        k == U - 1 or units[k + 1][0] != units[k][0]
                    )

                if j >= 0:
                    if is_last(j):
                        finish_window_A(units[j][0])
                    if is_last(j - 2):
                        finish_window_B2(units[j - 2][0])
                    ustate.pop(j, None)
            if U > 0:
                wl_last = units[U - 1][0]
                if wl_last in fstate and "I2" not in fstate[wl_last]:
                    finish_window_B1(wl_last)
                if wl_last in fstate:
                    finish_window_B2(wl_last)

    nc.compile()
    return nc


_LAST_RESULT = None


def kernel(**inputs):
    """Full-input GATv2 forward on 8 TRN2 NeuronCores (dst-sharded)."""
    global _LAST_RESULT
    n_cores = 8
    in_maps, meta = preprocess(
        inputs["x"], inputs["edge_index"], inputs["edge_attr"],
        inputs["Wl"], inputs["Wr"], inputs["We"], inputs["att"],
        inputs["bias"], n_cores,
    )
    nc = build(meta)
    from concourse.bass_utils import run_bass_kernel_spmd

    res = run_bass_kernel_spmd(nc, in_maps, core_ids=list(range(n_cores)))
    _LAST_RESULT = res
    W = meta["W"]
    N = meta["ND"] * n_cores
    HC = meta["HC"]
    bin_nodes = meta["bin_nodes"]
    bin_nused = meta["bin_nused"]
    full = np.concatenate(
        [np.asarray(res.results[c]["out"]).reshape(W * P, HC) for c in range(n_cores)],
        axis=0,
    ).reshape(-1, P, HC)
    out = np.zeros((N, HC), np.float32)
    real = np.arange(P)[None, :] < bin_nused[:, None]
    out[bin_nodes[real]] = full[real]
    return np.ascontiguousarray(out.astype(np.float32))


# revision 5
# speedup vs baseline: 1.4658x; 1.0047x over previous
"""GATv2 kernel v4: software-pipelined pair stream.

Same math/staging as v3 (merged self-loops, LPT-balanced bins, bf16 tables,
sum(alpha)=1 correction) but restructured as one flat stream of subtile
pairs with stage offsets so every engine stays busy:
  step i:  PE z(i) | DVE exY(i-3) | PE scatter(i-3) | Act lrelu(i-1)
           | Pool att-mult(i-1) | DVE head-reduce(i-2) | Act exp(i-2)
Padding slots are masked by zeros in the S/Sea scatter tables (no vmask).
"""

import numpy as np
from contextlib import ExitStack

import concourse.bass as bass
import concourse.tile as tile
from concourse import bacc, mybir

F32 = mybir.dt.float32
BF16 = mybir.dt.bfloat16
P = 128
NEG = 0.2


def preprocess(x, edge_index, edge_attr, Wl, Wr, We, att, bias, n_cores):
    import ml_dtypes

    BF = ml_dtypes.bfloat16
    x = np.ascontiguousarray(np.asarray(x, np.float32))
    src = np.asarray(edge_index[0]).astype(np.int64)
    dst = np.asarray(edge_index[1]).astype(np.int64)
    ea = np.asarray(edge_attr, np.float32).reshape(-1)
    Wl = np.ascontiguousarray(np.asarray(Wl, np.float32))
    Wr = np.ascontiguousarray(np.asarray(Wr, np.float32))
    We = np.asarray(We, np.float32).reshape(-1)
    att = np.asarray(att, np.float32)
    bias = np.asarray(bias, np.float32).reshape(-1)

    N, F = x.shape
    HC = Wl.shape[1]
    H = HC // 32
    assert F == P
    ND = N // n_cores
    W = (ND + P - 1) // P
    NB = n_cores * W

    cnt = np.bincount(dst, minlength=N).astype(np.int64)
    sums = np.zeros(N, np.float64)
    np.add.at(sums, dst, ea)
    la = (sums / np.maximum(cnt, 1)).astype(np.float32)

    src_f = np.concatenate([src, np.arange(N, dtype=np.int64)])
    dst_f = np.concatenate([dst, np.arange(N, dtype=np.int64)])
    ea_f = np.concatenate([ea, la])
    deg = cnt + 1

    import heapq

    order = np.argsort(-deg, kind="stable")
    heap = [(0, b) for b in range(NB)]
    heapq.heapify(heap)
    slots_used = np.zeros(NB, np.int64)
    node_bin = np.zeros(N, np.int64)
    node_row = np.zeros(N, np.int64)
    stash = []
    for n in order:
        while True:
            load, b = heapq.heappop(heap)
            if slots_used[b] < P:
                break
            stash.append((load, b))
        node_bin[n] = b
        node_row[n] = slots_used[b]
        slots_used[b] += 1
        heapq.heappush(heap, (load + int(deg[n]), b))
        for it in stash:
            heapq.heappush(heap, it)
        stash.clear()

    binload = np.bincount(node_bin[dst_f], minlength=NB)
    T = int(np.ceil(binload.max() / P))

    ebin = node_bin[dst_f]
    eorder = np.argsort(ebin, kind="stable")
    src_s = src_f[eorder]
    dst_s = dst_f[eorder]
    ea_s = ea_f[eorder]
    ebin_s = ebin[eorder]
    starts = np.zeros(NB, np.int64)
    np.cumsum(binload[:-1], out=starts[1:])
    rank = np.arange(len(src_s)) - starts[ebin_s]
    t_of = rank // P
    p_of = rank % P

    core = ebin_s // W
    w_of = ebin_s % W

    slot_src = np.zeros((n_cores, W, T, P), np.int64)
    slot_dstn = np.zeros((n_cores, W, T, P), np.int64)
    slot_row = np.zeros((n_cores, W, T, P), np.int64)
    slot_ea = np.zeros((n_cores, W, T, P), np.float32)
    slot_valid = np.zeros((n_cores, W, T, P), np.float32)

    slot_src[core, w_of, t_of, p_of] = src_s
    slot_dstn[core, w_of, t_of, p_of] = dst_s
    slot_row[core, w_of, t_of, p_of] = node_row[dst_s]
    slot_ea[core, w_of, t_of, p_of] = ea_s
    slot_valid[core, w_of, t_of, p_of] = 1.0

    rows = np.arange(P).reshape(1, 1, 1, 1, P)
    onehot = (slot_row[:, :, :, :, None] == rows) & (
        slot_valid[:, :, :, :, None] > 0
    )
    s_all = onehot.astype(np.float32).transpose(0, 1, 3, 2, 4).reshape(
        n_cores, W, P, T * P
    )
    sea_all = (onehot * slot_ea[:, :, :, :, None]).astype(np.float32).transpose(
        0, 1, 3, 2, 4
    ).reshape(n_cores, W, P, T * P)
    earow = slot_ea.reshape(n_cores, W, T * P)

    bin_nodes = np.zeros((NB, P), np.int64)
    bin_nodes[node_bin, node_row] = np.arange(N)
    xw = x.T[:, bin_nodes.reshape(-1)].reshape(P, NB, P)

    xT = x.T
    att_rep2 = np.tile(att.reshape(1, HC), (P, 2))
    we_row = We.reshape(1, HC)
    we_rep = np.broadcast_to(We.reshape(1, HC), (P, HC))
    bias_neg = (-bias).reshape(1, HC)
    ones_r = np.ones((1, P), np.float32)

    in_maps = []
    for c in range(n_cores):
        flat_s = slot_src[c].reshape(-1)
        flat_d = slot_dstn[c].reshape(-1)
        in_maps.append(
            dict(
                xTe=np.ascontiguousarray(xT[:, flat_s].astype(BF)),
                xTr=np.ascontiguousarray(xT[:, flat_d].astype(BF)),
                s_all=np.ascontiguousarray(s_all[c].astype(BF)),
                sea_all=np.ascontiguousarray(sea_all[c].astype(BF)),
                earow=np.ascontiguousarray(earow[c].astype(BF)),
                xw=np.ascontiguousarray(
                    xw[:, c * W : (c + 1) * W, :].transpose(1, 0, 2).astype(BF)
                ),
                Wl=Wl.astype(BF),
                Wr=Wr.astype(BF),
                we_row=np.ascontiguousarray(we_row.astype(BF)),
                we_rep=np.ascontiguousarray(we_rep.astype(np.float32)),
                att_rep2=np.ascontiguousarray(att_rep2.astype(BF)),
                bias_neg=np.ascontiguousarray(bias_neg.astype(BF)),
                ones_r=np.ascontiguousarray(ones_r.astype(BF)),
            )
        )
    meta = dict(
        W=W, T=T, HC=HC, H=H, ND=ND, NDpad=W * P, n_cores=n_cores,
        bin_nodes=bin_nodes, bin_nused=slots_used.copy(),
    )
    return in_maps, meta


def build(meta):
    W, T, HC, H = meta["W"], meta["T"], meta["HC"], meta["H"]
    NDpad = meta["NDpad"]
    WT = W * T

    nc = bacc.Bacc("TRN2", target_bir_lowering=False, debug=False)

    xTe = nc.dram_tensor("xTe", [P, WT * P], BF16, kind="ExternalInput")
    xTr = nc.dram_tensor("xTr", [P, WT * P], BF16, kind="ExternalInput")
    s_all = nc.dram_tensor("s_all", [W, P, T * P], BF16, kind="ExternalInput")
    sea_all = nc.dram_tensor("sea_all", [W, P, T * P], BF16, kind="ExternalInput")
    earow = nc.dram_tensor("earow", [W, T * P], BF16, kind="ExternalInput")
    xw = nc.dram_tensor("xw", [W, P, P], BF16, kind="ExternalInput")
    Wl = nc.dram_tensor("Wl", [P, HC], BF16, kind="ExternalInput")
    Wr = nc.dram_tensor("Wr", [P, HC], BF16, kind="ExternalInput")
    we_row = nc.dram_tensor("we_row", [1, HC], BF16, kind="ExternalInput")
    we_rep = nc.dram_tensor("we_rep", [P, HC], F32, kind="ExternalInput")
    att_rep2 = nc.dram_tensor("att_rep2", [P, 2 * HC], BF16, kind="ExternalInput")
    bias_neg = nc.dram_tensor("bias_neg", [1, HC], BF16, kind="ExternalInput")
    ones_r = nc.dram_tensor("ones_r", [1, P], BF16, kind="ExternalInput")
    out = nc.dram_tensor("out", [NDpad, HC], F32, kind="ExternalOutput")

    n_pair = T // 2
    n_tail = T - 2 * n_pair
    ppw = n_pair + n_tail  # pair-units per window
    units = []
    for w in range(W):
        for pi in range(ppw):
            t0 = 2 * pi
            cnt = 2 if pi < n_pair else 1
            units.append((w, t0, cnt))
    U = len(units)
    DEPTH = 4  # scatter trails z by this many units

    with tile.TileContext(nc) as tc, ExitStack() as ctx:
        cpool = ctx.enter_context(tc.tile_pool(name="cpool", bufs=1))
        wl_t = cpool.tile([P, HC], BF16)
        nc.scalar.dma_start(wl_t[:], Wl[:, :])
        wr_t = cpool.tile([P, HC], BF16)
        nc.scalar.dma_start(wr_t[:], Wr[:, :])
        werow_t = cpool.tile([1, HC], BF16)
        nc.scalar.dma_start(werow_t[:], we_row[:, :])
        werep_t = cpool.tile([P, HC], F32)
        nc.scalar.dma_start(werep_t[:], we_rep[:, :])
        attrep2_t = cpool.tile([P, 2 * HC], BF16)
        nc.scalar.dma_start(attrep2_t[:], att_rep2[:, :])
        biasneg_row = cpool.tile([1, HC], BF16)
        nc.scalar.dma_start(biasneg_row[:], bias_neg[:, :])
        ones_row = cpool.tile([1, P], BF16)
        nc.scalar.dma_start(ones_row[:], ones_r[:, :])

        with tc.tile_pool(name="win", bufs=2) as winp, tc.tile_pool(
            name="sub", bufs=6
        ) as subp, tc.tile_pool(name="xsps", bufs=5, space="PSUM") as xsps, tc.tile_pool(
            name="aggps", bufs=1, space="PSUM"
        ) as aggps, tc.tile_pool(name="denps", bufs=1, space="PSUM") as denps, tc.tile_pool(
            name="aeaps", bufs=1, space="PSUM"
        ) as aeaps, tc.tile_pool(name="post", bufs=2) as postp:
            wstate = {}  # w -> dict of window tiles
            ustate = {}  # i -> dict of per-unit tiles

            def open_window(w):
                st = {}
                st["xe"] = winp.tile([P, T * P], BF16, tag="xe", name="xe_t")
                nc.sync.dma_start(st["xe"][:], xTe[:, w * T * P : (w + 1) * T * P])
                st["xr"] = winp.tile([P, T * P], BF16, tag="xr", name="xr_t")
                nc.sync.dma_start(st["xr"][:], xTr[:, w * T * P : (w + 1) * T * P])
                st["ear"] = winp.tile([1, T * P], BF16, tag="ear", name="ear_t")
                nc.sync.dma_start(st["ear"][:], earow[w : w + 1, :])
                st["S"] = winp.tile([P, T * P], BF16, tag="S", name="S_t")
                nc.sync.dma_start(st["S"][:], s_all[w, :, :])
                st["Sea"] = winp.tile([P, T * P], BF16, tag="Sea", name="Sea_t")
                nc.sync.dma_start(st["Sea"][:], sea_all[w, :, :])
                st["xw"] = winp.tile([P, P], BF16, tag="xw", name="xw_t")
                nc.sync.dma_start(st["xw"][:], xw[w, :, :])
                st["agg"] = aggps.tile([P, 2 * HC], F32, tag="agg", name="agg")
                st["den"] = denps.tile([P, H], F32, tag="den", name="den")
                st["aea"] = aeaps.tile([P, H], F32, tag="aea", name="aea")
                return st

            def stage_z(i):
                w, t0, cnt = units[i]
                if w not in wstate:
                    wstate[w] = open_window(w)
                st = wstate[w]
                xs = xsps.tile([P, 2 * HC], F32, tag="xs")
                for k in range(cnt):
                    t = t0 + k
                    reg = xs[:, k * HC : (k + 1) * HC]
                    xe_t = st["xe"][:, t * P : (t + 1) * P]
                    xr_t = st["xr"][:, t * P : (t + 1) * P]
                    nc.tensor.matmul(reg, xe_t, wl_t[:], start=True, stop=False)
                    nc.tensor.matmul(reg, xr_t, wr_t[:], start=False, stop=False)
                    nc.tensor.matmul(
                        reg, st["ear"][0:1, t * P : (t + 1) * P], werow_t[:],
                        start=False, stop=True,
                    )
                ustate[i] = dict(xs=xs)

            def stage_Ltm(i):
                w, t0, cnt = units[i]
                u = ustate[i]
                u["presplit"] = cnt == 1
                L2 = subp.tile([P, 2 * HC], BF16, tag="L2")
                nc.scalar.activation(
                    out=L2[:, 0 : cnt * HC], in_=u["xs"][:, 0 : cnt * HC],
                    func=mybir.ActivationFunctionType.Prelu,
                    bias=0.0, scale=1.0, alpha=NEG,
                )
                tm2 = subp.tile([P, 2 * HC], BF16, tag="tm2")
                if u["presplit"] and cnt == 2:
                    for k in range(cnt):
                        nc.gpsimd.tensor_tensor(
                            out=tm2[:, k * HC : (k + 1) * HC],
                            in0=L2[:, k * HC : (k + 1) * HC],
                            in1=attrep2_t[:, k * HC : (k + 1) * HC],
                            op=mybir.AluOpType.mult,
                        )
                else:
                    nc.gpsimd.tensor_tensor(
                        out=tm2[:, 0 : cnt * HC], in0=L2[:, 0 : cnt * HC],
                        in1=attrep2_t[:, 0 : cnt * HC], op=mybir.AluOpType.mult,
                    )
                u["tm2"] = tm2
                if u["presplit"]:
                    tmv = tm2[:, 0 : cnt * HC].rearrange(
                        "p (u h c) -> p u h c", u=cnt, c=32
                    )
                    pm = subp.tile([P, HC], BF16, tag="pm")
                    nc.gpsimd.tensor_tensor(
                        out=pm[:, 0 : cnt * HC // 2].rearrange(
                            "p (u h c) -> p u h c", u=cnt, c=16
                        ),
                        in0=tmv[:, :, :, 0:16],
                        in1=tmv[:, :, :, 16:32],
                        op=mybir.AluOpType.add,
                    )
                    u["pm"] = pm

            def stage_redexp(i):
                w, t0, cnt = units[i]
                u = ustate[i]
                lg = subp.tile([P, 2 * H], F32, tag="lg")
                if u["presplit"]:
                    red_in = u["pm"][:, 0 : cnt * HC // 2].rearrange(
                        "p (u h c) -> p u h c", u=cnt, c=16
                    )
                else:
                    red_in = u["tm2"][:, 0 : cnt * HC].rearrange(
                        "p (u h c) -> p u h c", u=cnt, c=32
                    )
                nc.vector.tensor_reduce(
                    out=lg[:, 0 : cnt * H],
                    in_=red_in,
                    axis=mybir.AxisListType.X,
                    op=mybir.AluOpType.add,
                )
                ext = subp.tile([P, 2 * H], BF16, tag="ext")
                nc.scalar.activation(
                    out=ext[:, 0 : cnt * H], in_=lg[:, 0 : cnt * H],
                    func=mybir.ActivationFunctionType.Exp,
                )
                u["ext"] = ext

            def stage_scatter(i):
                w, t0, cnt = units[i]
                u = ustate[i]
                st = wstate[w]
                exy = subp.tile([P, 2 * HC], BF16, tag="exy")
                nc.vector.tensor_tensor(
                    out=exy[:, 0 : cnt * HC].rearrange(
                        "p (u h c) -> p u h c", u=cnt, c=32
                    ),
                    in0=u["xs"][:, 0 : cnt * HC].rearrange(
                        "p (u h c) -> p u h c", u=cnt, c=32
                    ),
                    in1=u["ext"][:, 0 : cnt * H]
                    .rearrange("p (u h) -> p u h", u=cnt)
                    .unsqueeze(3)
                    .to_broadcast([P, cnt, H, 32]),
                    op=mybir.AluOpType.mult,
                )
                for k in range(cnt):
                    t = t0 + k
                    S_t = st["S"][:, t * P : (t + 1) * P]
                    nc.tensor.matmul(
                        st["agg"][:, 0:HC], S_t, exy[:, k * HC : (k + 1) * HC],
                        start=(t == 0), stop=(t == T - 1),
                    )
                    nc.tensor.matmul(
                        st["den"][:], S_t,
                        u["ext"][:, k * H : (k + 1) * H],
                        start=(t == 0), stop=(t == T - 1),
                    )
                    nc.tensor.matmul(
                        st["aea"][:],
                        st["Sea"][:, t * P : (t + 1) * P],
                        u["ext"][:, k * H : (k + 1) * H],
                        start=(t == 0), stop=(t == T - 1),
                    )
                del u["xs"]

            fstate = {}

            def finish_window_A(w):
                st = wstate[w]
                xrw = st["agg"][:, HC : 2 * HC]
                nc.tensor.matmul(
                    xrw, st["xw"][:], wr_t[:], start=True, stop=False
                )
                nc.tensor.matmul(
                    xrw, ones_row[:], biasneg_row[:], start=False, stop=True
                )
                rc = postp.tile([P, H], F32, tag="rc")
                nc.vector.reciprocal(rc[:], st["den"][:])
                sh = postp.tile([P, H], F32, tag="sh")
                nc.vector.tensor_tensor(
                    out=sh[:], in0=st["aea"][:], in1=rc[:],
                    op=mybir.AluOpType.mult,
                )
                G = postp.tile([P, HC], F32, tag="G")
                nc.vector.tensor_tensor(
                    out=G[:].rearrange("p (h c) -> p h c", c=32),
                    in0=st["agg"][:, 0:HC].rearrange("p (h c) -> p h c", c=32),
                    in1=rc[:].unsqueeze(2).to_broadcast([P, H, 32]),
                    op=mybir.AluOpType.mult,
                )
                Hh = postp.tile([P, HC], F32, tag="Hh")
                nc.vector.tensor_tensor(
                    out=Hh[:].rearrange("p (h c) -> p h c", c=32),
                    in0=werep_t[:].rearrange("p (h c) -> p h c", c=32),
                    in1=sh[:].unsqueeze(2).to_broadcast([P, H, 32]),
                    op=mybir.AluOpType.mult,
                )
                fstate[w] = dict(G=G, Hh=Hh)

            def finish_window_B1(w):
                fs = fstate[w]
                I2 = postp.tile([P, HC], F32, tag="I2")
                nc.vector.scalar_tensor_tensor(
                    out=I2[:], in0=fs["Hh"][:], scalar=-1.0, in1=fs["G"][:],
                    op0=mybir.AluOpType.mult, op1=mybir.AluOpType.add,
                )
                fs["I2"] = I2

            def finish_window_B2(w):
                st = wstate.pop(w)
                fs = fstate.pop(w)
                xrw = st["agg"][:, HC : 2 * HC]
                out1 = postp.tile([P, HC], F32, tag="out1")
                nc.vector.scalar_tensor_tensor(
                    out=out1[:], in0=xrw, scalar=-1.0, in1=fs["I2"][:],
                    op0=mybir.AluOpType.mult, op1=mybir.AluOpType.add,
                )
                nc.sync.dma_start(out[w * P : (w + 1) * P, :], out1[:])

            for i in range(U + DEPTH):
                if i < U:
                    stage_z(i)
                j = i - DEPTH
                if j >= 0:
                    stage_scatter(j)
                    if 0 <= j - 1 < U and (
                        j - 1 == U - 1 or units[j][0] != units[j - 1][0]
                    ):
                        finish_window_B1(units[j - 1][0])
                if i - 1 >= 0 and i - 1 < U:
                    stage_Ltm(i - 1)
                if i - 2 >= 0 and i - 2 < U:
                    stage_redexp(i - 2)
                def is_last(k):
                    return 0 <= k < U and (
                        k == U - 1 or units[k + 1][0] != units[k][0]
                    )

                if j >= 0:
                    if is_last(j):
                        finish_window_A(units[j][0])
                    if is_last(j - 2):
                        finish_window_B2(units[j - 2][0])
                    ustate.pop(j, None)
            if U > 0:
                wl_last = units[U - 1][0]
                if wl_last in fstate and "I2" not in fstate[wl_last]:
                    finish_window_B1(wl_last)
                if wl_last in fstate:
                    finish_window_B2(wl_last)

    nc.compile()
    return nc


_LAST_RESULT = None


def kernel(**inputs):
    """Full-input GATv2 forward on 8 TRN2 NeuronCores (dst-sharded)."""
    global _LAST_RESULT
    n_cores = 8
    in_maps, meta = preprocess(
        inputs["x"], inputs["edge_index"], inputs["edge_attr"],
        inputs["Wl"], inputs["Wr"], inputs["We"], inputs["att"],
        inputs["bias"], n_cores,
    )
    nc = build(meta)
    from concourse.bass_utils import run_bass_kernel_spmd

    res = run_bass_kernel_spmd(nc, in_maps, core_ids=list(range(n_cores)))
    _LAST_RESULT = res
    W = meta["W"]
    N = meta["ND"] * n_cores
    HC = meta["HC"]
    bin_nodes = meta["bin_nodes"]
    bin_nused = meta["bin_nused"]
    full = np.concatenate(
        [np.asarray(res.results[c]["out"]).reshape(W * P, HC) for c in range(n_cores)],
        axis=0,
    ).reshape(-1, P, HC)
    out = np.zeros((N, HC), np.float32)
    real = np.arange(P)[None, :] < bin_nused[:, None]
    out[bin_nodes[real]] = full[real]
    return np.ascontiguousarray(out.astype(np.float32))
